# revision 38
# baseline (speedup 1.0000x reference)
"""Trainium2 Bass kernel for nn_Graph_module_net_0_18631568130110.

GNN message-passing block (two chained masked-sigmoid attention + grouped-conv
layers with a LayerNorm). Shapes: B=8, N=1024, C=MID=OUT=256, h=4, groups=4.

Math simplifications (exact):
  - The reference's relu(cosine)/top-k "present" column mask is the identity
    (unit diagonal of the cosine matrix puts every column in its own row's
    top-k, and the scatter is global), so it is omitted.
  - masks = masks_roi * score_mask[j]: rows j with score_mask[j]==0 contribute
    nothing to the attention message. The kernel compacts the j axis to the
    surviving ~N/2 indices (host-computed index list; selection only, no
    arithmetic on host) and pads to a multiple of 128.

Sharding: data-parallel over batch B across the 8 cores.

Layout strategy per core (J = padded count of surviving j):
  - attention built in [j(part), i(free)] layout; ACT (the only sigmoid
    engine) is the bottleneck at ~1038ns per [128,1024] sigmoid, so ACT does
    almost nothing else: one sigmoid per (j-chunk, head) with per-partition
    bias qt over krow_h = broadcast(kt[:,h]) tiles.
  - krow broadcast tiles are built by a DRAM round-trip (ktb -> internal DRAM
    scratch -> stride-0 partition-broadcast DMAs), zero engine time.
  - conv biases are seeded into PSUM by rank-1 PE matmuls (ones x cb row), so
    conv epilogues are pure relu copies; they run on ACT inside its forced
    idle windows (kernel head / block transition, while krow DMAs fly).
  - the 0.25-scaled transposed conv-output message stationaries (ocqT) are
    computed directly by PE matmuls against the gathered inputs (xq / out1q)
    instead of gather+transpose+copy of the full conv output.
  - all big operands are bf16: PE matmuls 4x faster than fp32, DVE
    elementwise 2x; sigmoid/mask products stay within the 2e-2 tolerance.
  - elementwise mask-mul work is split between DVE and GpSimd.
"""

import numpy as np
from contextlib import ExitStack

import concourse.bass as bass
import concourse.bacc as bacc
import concourse.tile as tile
from concourse import mybir
from concourse.bass_utils import run_bass_kernel_spmd
from concourse.masks import make_identity

F32 = mybir.dt.float32
BF = mybir.dt.bfloat16
U16 = mybir.dt.uint16
AF = mybir.ActivationFunctionType
ALU = mybir.AluOpType
NPBF = mybir.dt.np(BF)

N = 1024
C = 256
H = 4
P = 128
NC_ = N // P          # 8 chunks of 128 nodes
GC = C // P           # 2 partition chunks of channels
EPS_LN = 1e-6

SPK_CB = N + 1024 + 16   # offset of cb rows inside spk (row 0)

_CACHE = {}
TRACE = False


def _build_program(J, ln_trivial):
    JC = J // P
    nc = bacc.Bacc(None, target_bir_lowering=False)
    d_xT = nc.dram_tensor("xT", [C, N], BF, kind="ExternalInput")
    d_xq = nc.dram_tensor("xq", [C, J], BF, kind="ExternalInput")
    d_roiTq = nc.dram_tensor("roiTq", [J, N], BF, kind="ExternalInput")
    d_jidx = nc.dram_tensor("jidx", [P, J // 16], U16, kind="ExternalInput")
    d_smb = nc.dram_tensor("smb", [N], BF, kind="ExternalInput")
    d_bfp = nc.dram_tensor("bfp", [P, 288], BF, kind="ExternalInput")
    d_f32p = nc.dram_tensor("f32p", [P, 20], F32, kind="ExternalInput")
    d_spka = nc.dram_tensor("spka", [2, N + 16 + 512], BF, kind="ExternalInput")
    d_spkq = nc.dram_tensor("spkq", [2, 1024], BF, kind="ExternalInput")
    d_lng = nc.dram_tensor("ln_g", [C], F32, kind="ExternalInput")
    d_lnb = nc.dram_tensor("ln_b", [C], F32, kind="ExternalInput")
    d_kscr = [nc.dram_tensor(f"kscr{b}", [H, N], BF, kind="Internal")
              for b in range(2)]
    d_out = nc.dram_tensor("out", [N, C], BF, kind="ExternalOutput")

    with tile.TileContext(nc) as tc, ExitStack() as ctx:
        consts = ctx.enter_context(tc.tile_pool(name="consts", bufs=1))
        persist = ctx.enter_context(tc.tile_pool(name="persist", bufs=1))
        small = ctx.enter_context(tc.tile_pool(name="small", bufs=1))
        sp = ctx.enter_context(tc.tile_pool(name="sp", bufs=8))
        ap_ = ctx.enter_context(tc.tile_pool(name="ap", bufs=6))
        lnp = ctx.enter_context(tc.tile_pool(name="lnp", bufs=6))
        pm = ctx.enter_context(tc.tile_pool(name="pm", bufs=1, space="PSUM"))
        pk = ctx.enter_context(tc.tile_pool(name="pk", bufs=2, space="PSUM"))
        dma = nc.default_dma_engine   # SP queue

        # ---------------- constants / weights ----------------
        identity = consts.tile([P, P], BF)
        make_identity(nc, identity[:])
        pdum = pk.tile([32, 32], BF, name="pdum", tag="w")
        nc.tensor.transpose(pdum, identity[0:32, 0:32], identity[0:32, 0:32])
        epst = consts.tile([P, 1], F32)
        nc.vector.memset(epst, EPS_LN)

        def load(pool, shape, dt, src, nm, eng=dma):
            t = pool.tile(shape, dt, name=nm, tag=nm)
            eng.dma_start(t, src)
            return t

        # SP queue: xT0 first (gates ktb1), then the small weight packs.
        # Pool queue: xT1 / xq / spkq in parallel.
        xT0 = load(persist, [P, N], BF, d_xT[0:P, :], "xT0")
        spka = load(consts, [2, N + 16 + 512], BF, d_spka[:], "spka",
                    eng=(dma if _CACHE.get("nowarm") else nc.scalar))
        spatT = spka[:, 0:N]
        w1sT = spka[:, N:N + 8]
        w2sT = spka[:, N + 8:N + 16]
        cb1row = spka[0:1, N + 16:N + 272]
        cb2row = spka[0:1, N + 272:N + 528]
        bfp = load(consts, [P, 288], BF, d_bfp[:], "bfp")
        w1T = [bfp[:, 0:8], bfp[:, 8:16]]          # cols 0:4 qT, 4:8 kT
        w2T = [bfp[:, 16:24], bfp[:, 24:32]]
        cw1T = [bfp[:, 32:96], bfp[:, 96:160]]
        cw2T = [bfp[:, 160:224], bfp[:, 224:288]]
        f32p = load(consts, [P, 20], F32, d_f32p[:], "f32p")
        cb1t, cb2t, smt = f32p[:, 0:2], f32p[:, 2:4], f32p[:, 4:12]
        brep = {"1": f32p[:, 12:16], "2": f32p[:, 16:20]}
        jidx = load(consts, [P, J // 16], U16, d_jidx[:], "jidx")
        xT1 = load(persist, [P, N], BF, d_xT[P:C, :], "xT1", eng=nc.gpsimd)
        xT = [xT0, xT1]
        xq = [load(persist, [P, J], BF, d_xq[cc * P:(cc + 1) * P, :], f"xq{cc}",
                   eng=nc.gpsimd) for cc in range(GC)]
        spkq = load(consts, [2, 1024], BF, d_spkq[:], "spkq", eng=nc.gpsimd)
        spatq = spkq[:, 0:J]
        ones = consts.tile([1, 512], BF, name="ones", tag="ones")
        nc.vector.memset(ones, 1.0)
        roiTq = [persist.tile([P, N], BF, name=f"roiTq{jq}", tag=f"roiTq{jq}")
                 for jq in range(JC)]
        for jq in range(2):
            dma.dma_start(roiTq[jq], d_roiTq[jq * P:(jq + 1) * P, :])

        sigwarm = consts.tile([P, 1], F32, name="sigwarm", tag="sigwarm")
        nc.scalar.activation(sigwarm, epst, AF.Sigmoid)

        def bcast_row(dvec, name):
            t = consts.tile([P, C], F32, tag=name)
            _dv = dvec[:]
            ap_b = bass.AP(tensor=_dv.tensor, offset=_dv.offset,
                           ap=[[0, P]] + list(_dv.ap))
            dma.dma_start(t, ap_b)
            return t
        if not ln_trivial:
            lngrow = bcast_row(d_lng, "lngrow")
            lnbrow = bcast_row(d_lnb, "lnbrow")



        # ---------------- per-block emitters ----------------
        def emit_qt(xq_src, spq, wT, wsT, tag):
            """qt[jq] tiles [128, H] f32 (per-partition sigmoid bias)."""
            qts = []
            for jq in range(JC):
                pq = pk.tile([P, H], F32, name="pq", tag="w")
                sl = slice(jq * P, (jq + 1) * P)
                for cc in range(GC):
                    nc.tensor.matmul(pq, xq_src[cc][:, sl], wT[cc][:, 0:H],
                                     start=(cc == 0), stop=False)
                nc.tensor.matmul(pq, spq[:, sl], wsT[:, 0:H], start=False, stop=True)
                qt = small.tile([P, H], F32, name=f"qt{tag}{jq}", tag=f"qt{tag}{jq}")
                nc.vector.tensor_add(qt, pq, brep[tag])
                qts.append(qt)
            return qts

        def emit_krow(x_src, spT, wT, wsT, bi, tag, q01, q23):
            """krow[h] [128, N] bf16 = broadcast(kt[:,h]) via DRAM round-trip.

            q01: queue for the scratch write + h0/h1 broadcasts (on the krow
            critical path); q23: queue for the h2/h3 broadcasts."""
            ktb = small.tile([H, N], BF, name=f"ktb{tag}", tag=f"ktb{tag}")
            for half in range(2):
                pkt = pk.tile([H, 512], F32, name="pkt", tag="w")
                sl = slice(half * 512, (half + 1) * 512)
                for cc in range(GC):
                    nc.tensor.matmul(pkt, wT[cc][:, H:2 * H],
                                     x_src[cc][:, sl], start=(cc == 0), stop=False)
                nc.tensor.matmul(pkt, wsT[:, H:2 * H], spT[:, sl],
                                 start=False, stop=True)
                nc.vector.tensor_copy(ktb[:, sl], pkt)
            # head 0 skips any broadcast: replicate ktb row 0 down the
            # partitions with a rank-1 matmul (ones x row) straight into PSUM
            # (rows 1-3 are not at base partition 0, so only h0 can do this).
            kr0 = pm.tile([P, N], F32, name=f"krp{tag}", tag="krp")
            for half in range(2):
                sl = slice(half * 512, (half + 1) * 512)
                nc.tensor.matmul(kr0[:, sl], ones[:, 0:P], ktb[0:1, sl],
                                 start=True, stop=True, skip_group_check=True)
            # rows 1-3: bounce into partition 0 of a [1, 3N] tile (one DMA),
            # then GpSimd partition_broadcast (reads partition 0 only)
            kr0b = small.tile([1, 3 * N], BF, name=f"kr0{tag}", tag=f"kr0{tag}")
            _k0 = kr0b[:]
            kr0_dst = bass.AP(tensor=_k0.tensor, offset=_k0.offset,
                              ap=[list(_k0.ap[0]), [N, 3], [1, N]])
            q01.dma_start(kr0_dst, ktb[1:4, :])
            krows = [kr0]
            for h in range(1, H):
                kr = persist.tile([P, N], BF, name=f"krow{tag}{h}", tag=f"krow{h}")
                nc.gpsimd.partition_broadcast(kr, kr0b[0:1, (h - 1) * N:h * N])
                krows.append(kr)
            return krows

        def emit_conv(src, cwT, cbrow, tag, act_epi=True):
            """oc [cc][128, N] bf16 = relu(grouped 1x1 conv + bias).

            Bias is seeded into PSUM by a rank-1 matmul; epilogue is a pure
            relu copy on ACT (fills ACT's forced idle windows)."""
            oc = [persist.tile([P, N], BF, name=f"oc{tag}{cc}", tag=f"oc{cc}",
                               bufs=1) for cc in range(GC)]
            for cc in range(GC):
                for half in range(2):
                    sl = slice(half * 512, (half + 1) * 512)
                    pc = pk.tile([P, 512], F32, name="pconv", tag="w")
                    nc.tensor.matmul(pc, cbrow[:, cc * P:(cc + 1) * P], ones,
                                     start=True, stop=False, skip_group_check=True)
                    for ro in (0, 64):
                        nc.tensor.matmul(pc[ro:ro + 64, :], cwT[cc][ro:ro + 64, :],
                                         src[cc][ro:ro + 64, sl],
                                         start=False, stop=True,
                                         skip_group_check=True)
                    if act_epi:
                        nc.scalar.activation(oc[cc][:, sl], pc, AF.Relu)
                    else:
                        nc.vector.tensor_scalar(oc[cc][:, sl], pc, 0.0, 0.0,
                                                ALU.max, ALU.bypass)
            return oc

        def emit_cqT(stq, cwT, cbrow, tag):
            """[jq][128, 256] bf16 = 0.25*relu(conv(stq)+cb)^T at gathered j.

            stq: gathered inputs [cc][128, J] (channels on partitions).
            Output columns are conv channels; rows are j within the chunk."""
            outs = []
            for jq in range(JC):
                pcq = pk.tile([P, C], F32, name="pcq", tag="w")
                nc.tensor.matmul(pcq, ones[:, 0:P], cbrow,
                                 start=True, stop=False, skip_group_check=True)
                for g in range(4):
                    cc, ro = g // 2, (g % 2) * 64
                    nc.tensor.matmul(pcq[:, g * 64:(g + 1) * 64],
                                     stq[cc][ro:ro + 64, jq * P:(jq + 1) * P],
                                     cwT[cc][ro:ro + 64, :],
                                     start=False, stop=True,
                                     skip_group_check=True)
                t = persist.tile([P, C], BF, name=f"cqT{tag}{jq}",
                                 tag=f"cqT{tag}{jq}", bufs=1)
                nc.vector.tensor_scalar(t, pcq, 0.25, 0.0, ALU.mult, ALU.max)
                outs.append(t)
            return outs

        def emit_gather(src, tag):
            """[cc][128, J] bf16 = src[:, jlist] via GpSimd indirect copy."""
            out = [persist.tile([P, J], BF, name=f"g{tag}{cc}", tag=f"g{tag}{cc}",
                                bufs=1) for cc in range(GC)]
            for cc in range(GC):
                nc.gpsimd.indirect_copy(out[cc], src[cc], jidx, True)
            return out

        def emit_message(qts, krows, ocqT, tag, hook=None, warm=False,
                         transposed=False):
            """pm [cc][half][128, 512] f32 psum = sum_j 0.25*oc^T*sig*roi.

            h-major: all jq chunks for head h before moving to h+1, so the
            cc0 psum tiles (heads 0,1) complete ~10us before the cc1 ones,
            and the krow_h broadcast DMAs get a full head-phase of slack.
            hook(h) is called after head h's matmuls are emitted."""
            tags = ["pma0", "pma1", "pmb0", "pmb1"]
            if transposed:
                # 4 tiles, each packing two i-chunks' [128, 256] msg^T
                pms = [[pm.tile([P, 512], F32, name=f"pm{tag}t{t}",
                                tag=tags[t])] for t in range(4)]
            else:
                pms = [[pm.tile([P, 512], F32, name=f"pm{tag}{cc}{hf}",
                                tag=tags[cc * 2 + hf]) for hf in range(2)]
                       for cc in range(GC)]
            for h in range(H):
                for jq in range(JC):
                    s = sp.tile([P, N], BF, name="sig", tag="sig")
                    nc.scalar.activation(s, krows[h], AF.Sigmoid,
                                         bias=qts[jq][:, h:h + 1])
                    a = ap_.tile([P, N], BF, name="attn", tag="attn")
                    eng = nc.vector if h in (0, 3) else nc.gpsimd
                    eng.tensor_mul(a, s, roiTq[jq])
                    if warm and not _CACHE.get("nowarm") and h == H - 1 and jq == JC - 1:
                        sqw = consts.tile([P, 1], F32, name="sqwarm",
                                          tag="sqwarm")
                        nc.scalar.activation(sqw, s[:, 0:1], AF.Sqrt)
                    if transposed:
                        # a as stationary: psum accumulates msg^T [i, ch].
                        # Tiles 0/1 hold the ch 0:128 blocks (heads 0,1) for
                        # ics 0-3 / 4-7; tiles 2/3 the ch 128:256 blocks —
                        # so early reads never touch a still-accumulating
                        # tile (real PSUM forbids read-during-accumulate).
                        for ic in range(NC_):
                            t = (0 if h < 2 else 2) + ic // 4
                            co = (ic % 4) * P + (h % 2) * 64
                            nc.tensor.matmul(
                                pms[t][0][:, co:co + 64],
                                a[:, ic * P:(ic + 1) * P],
                                ocqT[jq][:, h * 64:(h + 1) * 64],
                                start=(jq == 0), stop=(jq == JC - 1),
                                skip_group_check=True)
                    else:
                        for half in range(2):
                            sl = slice(half * 512, (half + 1) * 512)
                            nc.tensor.matmul(
                                pms[h // 2][half][(h % 2) * 64:(h % 2) * 64 + 64, :],
                                ocqT[jq][:, h * 64:(h + 1) * 64], a[:, sl],
                                start=(jq == 0), stop=(jq == JC - 1),
                                skip_group_check=True)
                if hook is not None:
                    hook(h, pms)
            return pms

        # keep the PE busy from t~0 so the HAM clock gate opens before the
        # first real matmuls (cold PE runs at half speed); the pm psum tiles
        # are unused until the message phase and get cleared by start=True.
        pwarm = pm.tile([P, 512], F32, name="pm10", tag="pma0")
        for wi in range(10):
            nc.tensor.matmul(pwarm[:, 0:128], identity, identity,
                             start=True, stop=True, skip_group_check=True)

        # ================= block 1 =================
        krows1 = emit_krow(xT, spatT, w1T, w1sT, 0, "1", nc.gpsimd, dma)
        # stream the remaining roi chunks + grow during phase 1
        for jq in range(2, JC):
            dma.dma_start(roiTq[jq], d_roiTq[jq * P:(jq + 1) * P, :])
        _sm = d_smb[:]
        sm_b = bass.AP(tensor=_sm.tensor, offset=_sm.offset,
                       ap=[[0, P]] + list(_sm.ap))
        growd = consts.tile([P, N], BF, name="growd", tag="growd")
        dma.dma_start(growd, sm_b)
        qts1 = emit_qt(xq, spatq, w1T, w1sT, "1")
        o1c = emit_conv(xT, cw1T, cb1row, "1", act_epi=False)
        o1cqT = emit_cqT(xq, cw1T, cb1row, "1")
        grow = consts.tile([P, N], BF, name="grow", tag="grow")  # 1 + f/4
        nc.vector.tensor_scalar(grow, growd, -0.25, 1.25, ALU.mult, ALU.add)
        fqt = consts.tile([P, NC_], F32, name="fqt", tag="fqt")  # f/4 = .25-.25*sm
        nc.vector.tensor_scalar(fqt, smt, -0.25, 0.25, ALU.mult, ALU.add)
        # t1 = o1c * grow runs early (o1c and grow are ready at ~12us)
        t1 = [persist.tile([P, N], BF, name=f"fin1a{cc}", tag=f"fin1a{cc}", bufs=1)
              for cc in range(GC)]
        nc.vector.tensor_mul(t1[0], o1c[0], grow)
        nc.vector.tensor_mul(t1[1], o1c[1], grow)
        # out1 = t1 + pm1 (pm1 already holds msg/4); the cc0 half and its
        # gather run mid-phase via the h==1 hook (cc0 psums done after h1)
        out1 = [persist.tile([P, N], BF, name=f"out1{cc}", tag=f"out1{cc}")
                for cc in range(GC)]
        out1q = [persist.tile([P, J], BF, name=f"gx2{cc}", tag=f"gx2{cc}",
                              bufs=1) for cc in range(GC)]

        def b1hook(h, pms):
            # cc0 psums are final after h1: assemble out1[0] during the h2
            # phase (DVE); gather it during the h3 phase (Pool is free then,
            # h3 muls run on DVE)
            if h == 1:
                for hf in range(2):
                    sl = slice(hf * 512, (hf + 1) * 512)
                    nc.vector.scalar_tensor_tensor(out1[0][:, sl], pms[0][hf],
                                                   1.0, t1[0][:, sl],
                                                   ALU.mult, ALU.add)
            elif h == 2:
                nc.gpsimd.indirect_copy(out1q[0], out1[0], jidx, True)

        pms1 = emit_message(qts1, krows1, o1cqT, "1", hook=(b1hook if not _CACHE.get("nohook") else None))
        if _CACHE.get("nohook"):
            for hf in range(2):
                sl = slice(hf * 512, (hf + 1) * 512)
                nc.vector.scalar_tensor_tensor(out1[0][:, sl], pms1[0][hf],
                                               1.0, t1[0][:, sl],
                                               ALU.mult, ALU.add)
            nc.gpsimd.indirect_copy(out1q[0], out1[0], jidx, True)
        for hf in range(2):
            sl = slice(hf * 512, (hf + 1) * 512)
            nc.vector.scalar_tensor_tensor(out1[1][:, sl], pms1[1][hf], 1.0,
                                           t1[1][:, sl], ALU.mult, ALU.add)
        nc.gpsimd.indirect_copy(out1q[1], out1[1], jidx, True)

        # ================= block 2 =================
        krows2 = emit_krow(out1, spatT, w2T, w2sT, 1, "2", dma, nc.gpsimd)
        qts2 = emit_qt(out1q, spatq, w2T, w2sT, "2")
        o2c = emit_conv(out1, cw2T, cb2row, "2", act_epi=False)
        o2cqT = emit_cqT(out1q, cw2T, cb2row, "2")
        # o2c^T tiles for the final phase, built while messages run
        o2cTs = []
        for ic in range(NC_):
            ptsA = pk.tile([P, C], BF, name="ptA", tag="w")
            for cc in range(GC):
                nc.tensor.transpose(ptsA[:, cc * P:(cc + 1) * P],
                                    o2c[cc][:, ic * P:(ic + 1) * P], identity)
            t = persist.tile([P, C], BF, name=f"o2cTs{ic}", tag=f"o2cTs{ic}")
            nc.vector.tensor_copy(t, ptsA)
            o2cTs.append(t)
        # fold the f-term scale in ahead of time: foc = (f/4) * o2cT, so the
        # per-ic critical path in the finals is a plain bf16 add (2x mode)
        foc = []
        for ic in range(NC_):
            t = persist.tile([P, C], BF, name=f"foc{ic}", tag=f"foc{ic}")
            nc.gpsimd.tensor_scalar(t, o2cTs[ic], fqt[:, ic:ic + 1], 0.0,
                                    ALU.mult, ALU.bypass)
            foc.append(t)
        # split-cc finals: the ch 0:128 (heads 0,1) message psum columns
        # finish a full head-phase before ch 128:256, so half of every LN
        # chain runs during the h2/h3 sigmoid phases. Messages are psum-
        # resident in [i, ch] layout (transposed matmuls) — no copies or
        # transposes needed.
        vt = [persist.tile([P, C], BF, name=f"lnv{ic}", tag=f"lnv{ic}")
              for ic in range(NC_)]
        vstats = [persist.tile([P, 12], F32, name=f"lnst{ic}", tag=f"lnst{ic}")
                  for ic in range(NC_)]

        def final_early(h, pms):
            # heads 0/1 msg^T columns are final after h1: fold them into vt
            # and take partial bn stats while heads 2/3 run
            if h != 1 or _CACHE.get("nohook"):
                return
            for ic in range(NC_):
                co = (ic % 4) * P
                nc.vector.tensor_add(vt[ic][:, 0:P], foc[ic][:, 0:P],
                                     pms[ic // 4][0][:, co:co + P])
                nc.vector.bn_stats(vstats[ic][:, 0:6], vt[ic][:, 0:P])

        pms2 = emit_message(qts2, krows2, o2cqT, "2", hook=final_early,
                            warm=True, transposed=True)

        obuf = [persist.tile([P, 2 * C], BF, name=f"obuf{t}", tag=f"obuf{t}")
                for t in range(NC_ // 2)]
        for ic in range(NC_):
            co = (ic % 4) * P
            if _CACHE.get("nohook"):
                nc.vector.tensor_add(vt[ic][:, 0:P], foc[ic][:, 0:P],
                                     pms2[ic // 4][0][:, co:co + P])
                nc.vector.bn_stats(vstats[ic][:, 0:6], vt[ic][:, 0:P])
            nc.vector.tensor_add(vt[ic][:, P:C], foc[ic][:, P:C],
                                 pms2[2 + ic // 4][0][:, co:co + P])
            nc.vector.bn_stats(vstats[ic][:, 6:12], vt[ic][:, P:C])
            mv = lnp.tile([P, nc.vector.BN_AGGR_DIM], F32, name="lnmv", tag="lnmv")
            nc.vector.bn_aggr(mv, vstats[ic])
            rstd = lnp.tile([P, 1], F32, name="lnrstd", tag="lnrstd")
            nc.scalar.activation(rstd, mv[:, 1:2], AF.Sqrt, bias=epst)
            nc.vector.reciprocal(rstd, rstd)
            w = lnp.tile([P, C], BF, name="lnw", tag="lnw")
            nc.gpsimd.tensor_scalar(w, vt[ic], mv[:, 0:1], rstd,
                                    ALU.subtract, ALU.mult)
            if not ln_trivial:
                nc.gpsimd.tensor_mul(w, w, lngrow)
                nc.gpsimd.tensor_add(w, w, lnbrow)
            o = obuf[ic // 2][:, (ic % 2) * C:(ic % 2) * C + C]
            nc.gpsimd.tensor_add(o, w, o2cTs[ic])
            if ic % 2 == 1:
                _o = d_out[(ic - 1) * P:(ic + 1) * P, :]
                dst = bass.AP(tensor=_o.tensor, offset=_o.offset,
                              ap=[[C, P], [P * C, 2], [1, C]])
                dma.dma_start(dst, obuf[ic // 2])

    nc.finalize()
    return nc


# revision 46
# speedup vs baseline: 1.1863x; 1.1863x over previous
"""Trainium2 Bass kernel for nn_Graph_module_net_0_18631568130110.

GNN message-passing block (two chained masked-sigmoid attention + grouped-conv
layers with a LayerNorm). Shapes: B=8, N=1024, C=MID=OUT=256, h=4, groups=4.

Math simplifications (exact):
  - The reference's relu(cosine)/top-k "present" column mask is the identity
    (unit diagonal of the cosine matrix puts every column in its own row's
    top-k, and the scatter is global), so it is omitted.
  - masks = masks_roi * score_mask[j]: rows j with score_mask[j]==0 contribute
    nothing to the attention message. The kernel compacts the j axis to the
    surviving ~N/2 indices (host-computed index list; selection only, no
    arithmetic on host) and pads to a multiple of 128.

Sharding: data-parallel over batch B across the 8 cores.

Layout strategy per core (J = padded count of surviving j):
  - attention built in [j(part), i(free)] layout; ACT (the only sigmoid
    engine) is the bottleneck at ~1038ns per [128,1024] sigmoid, so ACT does
    almost nothing else: one sigmoid per (j-chunk, head) with per-partition
    bias qt over krow_h = broadcast(kt[:,h]) tiles.
  - krow broadcast tiles are built by a DRAM round-trip (ktb -> internal DRAM
    scratch -> stride-0 partition-broadcast DMAs), zero engine time.
  - conv biases are seeded into PSUM by rank-1 PE matmuls (ones x cb row), so
    conv epilogues are pure relu copies; they run on ACT inside its forced
    idle windows (kernel head / block transition, while krow DMAs fly).
  - the 0.25-scaled transposed conv-output message stationaries (ocqT) are
    computed directly by PE matmuls against the gathered inputs (xq / out1q)
    instead of gather+transpose+copy of the full conv output.
  - all big operands are bf16: PE matmuls 4x faster than fp32, DVE
    elementwise 2x; sigmoid/mask products stay within the 2e-2 tolerance.
  - elementwise mask-mul work is split between DVE and GpSimd.
"""

import numpy as np
from contextlib import ExitStack

import concourse.bass as bass
import concourse.bacc as bacc
import concourse.tile as tile
from concourse import mybir
from concourse.bass_utils import run_bass_kernel_spmd
from concourse.masks import make_identity

F32 = mybir.dt.float32
BF = mybir.dt.bfloat16
U16 = mybir.dt.uint16
AF = mybir.ActivationFunctionType
ALU = mybir.AluOpType
NPBF = mybir.dt.np(BF)

N = 1024
C = 256
H = 4
P = 128
NC_ = N // P          # 8 chunks of 128 nodes
GC = C // P           # 2 partition chunks of channels
EPS_LN = 1e-6

SPK_CB = N + 1024 + 16   # offset of cb rows inside spk (row 0)

_CACHE = {}
TRACE = False


def _build_program(J, ln_trivial):
    JC = J // P
    nc = bacc.Bacc(None, target_bir_lowering=False)
    d_xT = nc.dram_tensor("xT", [C, N], BF, kind="ExternalInput")
    d_xq = nc.dram_tensor("xq", [C, J], BF, kind="ExternalInput")
    d_roiTq = nc.dram_tensor("roiTq", [J, N], BF, kind="ExternalInput")
    d_jidx = nc.dram_tensor("jidx", [P, J // 16], U16, kind="ExternalInput")
    d_smb = nc.dram_tensor("smb", [N], BF, kind="ExternalInput")
    d_bfp = nc.dram_tensor("bfp", [P, 288], BF, kind="ExternalInput")
    d_f32p = nc.dram_tensor("f32p", [P, 20], F32, kind="ExternalInput")
    d_spka = nc.dram_tensor("spka", [2, N + 16 + 512], BF, kind="ExternalInput")
    d_spkq = nc.dram_tensor("spkq", [2, 1024], BF, kind="ExternalInput")
    d_lng = nc.dram_tensor("ln_g", [C], F32, kind="ExternalInput")
    d_lnb = nc.dram_tensor("ln_b", [C], F32, kind="ExternalInput")
    d_kscr = [nc.dram_tensor(f"kscr{b}", [H, N], BF, kind="Internal")
              for b in range(2)]
    d_out = nc.dram_tensor("out", [N, C], BF, kind="ExternalOutput")

    with tile.TileContext(nc) as tc, ExitStack() as ctx:
        consts = ctx.enter_context(tc.tile_pool(name="consts", bufs=1))
        persist = ctx.enter_context(tc.tile_pool(name="persist", bufs=1))
        small = ctx.enter_context(tc.tile_pool(name="small", bufs=1))
        sp = ctx.enter_context(tc.tile_pool(name="sp", bufs=8))
        ap_ = ctx.enter_context(tc.tile_pool(name="ap", bufs=6))
        lnp = ctx.enter_context(tc.tile_pool(name="lnp", bufs=6))
        pm = ctx.enter_context(tc.tile_pool(name="pm", bufs=1, space="PSUM"))
        pk = ctx.enter_context(tc.tile_pool(name="pk", bufs=2, space="PSUM"))
        dma = nc.default_dma_engine   # SP queue

        # ---------------- constants / weights ----------------
        identity = consts.tile([P, P], BF)
        make_identity(nc, identity[:])
        pdum = pk.tile([32, 32], BF, name="pdum", tag="w")
        nc.tensor.transpose(pdum, identity[0:32, 0:32], identity[0:32, 0:32])
        epst = consts.tile([P, 1], F32)
        nc.vector.memset(epst, EPS_LN)

        def load(pool, shape, dt, src, nm, eng=dma):
            t = pool.tile(shape, dt, name=nm, tag=nm)
            eng.dma_start(t, src)
            return t

        # SP queue: xT0 first (gates ktb1), then the small weight packs.
        # Pool queue: xT1 / xq / spkq in parallel.
        xT0 = load(persist, [P, N], BF, d_xT[0:P, :], "xT0")
        spka = load(consts, [2, N + 16 + 512], BF, d_spka[:], "spka",
                    eng=(dma if _CACHE.get("nowarm") else nc.scalar))
        spatT = spka[:, 0:N]
        w1sT = spka[:, N:N + 8]
        w2sT = spka[:, N + 8:N + 16]
        cb1row = spka[0:1, N + 16:N + 272]
        cb2row = spka[0:1, N + 272:N + 528]
        bfp = load(consts, [P, 288], BF, d_bfp[:], "bfp")
        w1T = [bfp[:, 0:8], bfp[:, 8:16]]          # cols 0:4 qT, 4:8 kT
        w2T = [bfp[:, 16:24], bfp[:, 24:32]]
        cw1T = [bfp[:, 32:96], bfp[:, 96:160]]
        cw2T = [bfp[:, 160:224], bfp[:, 224:288]]
        f32p = load(consts, [P, 20], F32, d_f32p[:], "f32p")
        cb1t, cb2t, smt = f32p[:, 0:2], f32p[:, 2:4], f32p[:, 4:12]
        brep = {"1": f32p[:, 12:16], "2": f32p[:, 16:20]}
        jidx = load(consts, [P, J // 16], U16, d_jidx[:], "jidx")
        xT1 = load(persist, [P, N], BF, d_xT[P:C, :], "xT1", eng=nc.gpsimd)
        xT = [xT0, xT1]
        xq = [load(persist, [P, J], BF, d_xq[cc * P:(cc + 1) * P, :], f"xq{cc}",
                   eng=nc.gpsimd) for cc in range(GC)]
        spkq = load(consts, [2, 1024], BF, d_spkq[:], "spkq", eng=nc.gpsimd)
        spatq = spkq[:, 0:J]
        ones = consts.tile([1, 512], BF, name="ones", tag="ones")
        nc.vector.memset(ones, 1.0)
        roiTq = [persist.tile([P, N], BF, name=f"roiTq{jq}", tag=f"roiTq{jq}")
                 for jq in range(JC)]
        for jq in range(2):
            dma.dma_start(roiTq[jq], d_roiTq[jq * P:(jq + 1) * P, :])

        sigwarm = consts.tile([P, 1], F32, name="sigwarm", tag="sigwarm")
        nc.scalar.activation(sigwarm, epst, AF.Sigmoid)

        def bcast_row(dvec, name):
            t = consts.tile([P, C], F32, tag=name)
            _dv = dvec[:]
            ap_b = bass.AP(tensor=_dv.tensor, offset=_dv.offset,
                           ap=[[0, P]] + list(_dv.ap))
            dma.dma_start(t, ap_b)
            return t
        if not ln_trivial:
            lngrow = bcast_row(d_lng, "lngrow")
            lnbrow = bcast_row(d_lnb, "lnbrow")



        # ---------------- per-block emitters ----------------
        def emit_qt(xq_src, spq, wT, wsT, tag):
            """qt[jq] tiles [128, H] f32 (per-partition sigmoid bias)."""
            qts = []
            for jq in range(JC):
                pq = pk.tile([P, H], F32, name="pq", tag="w")
                sl = slice(jq * P, (jq + 1) * P)
                for cc in range(GC):
                    nc.tensor.matmul(pq, xq_src[cc][:, sl], wT[cc][:, 0:H],
                                     start=(cc == 0), stop=False)
                nc.tensor.matmul(pq, spq[:, sl], wsT[:, 0:H], start=False, stop=True)
                qt = small.tile([P, H], F32, name=f"qt{tag}{jq}", tag=f"qt{tag}{jq}")
                nc.vector.tensor_add(qt, pq, brep[tag])
                qts.append(qt)
            return qts

        def emit_krow(x_src, spT, wT, wsT, bi, tag, q01, q23):
            """krow[h] [128, N] bf16 = broadcast(kt[:,h]) via DRAM round-trip.

            q01: queue for the scratch write + h0/h1 broadcasts (on the krow
            critical path); q23: queue for the h2/h3 broadcasts."""
            ktb = small.tile([H, N], BF, name=f"ktb{tag}", tag=f"ktb{tag}")
            for half in range(2):
                pkt = pk.tile([H, 512], F32, name="pkt", tag="w")
                sl = slice(half * 512, (half + 1) * 512)
                for cc in range(GC):
                    nc.tensor.matmul(pkt, wT[cc][:, H:2 * H],
                                     x_src[cc][:, sl], start=(cc == 0), stop=False)
                nc.tensor.matmul(pkt, wsT[:, H:2 * H], spT[:, sl],
                                 start=False, stop=True)
                nc.vector.tensor_copy(ktb[:, sl], pkt)
            # head 0 skips any broadcast: replicate ktb row 0 down the
            # partitions with a rank-1 matmul (ones x row) straight into PSUM
            # (rows 1-3 are not at base partition 0, so only h0 can do this).
            kr0 = pm.tile([P, N], F32, name=f"krp{tag}", tag="krp")
            for half in range(2):
                sl = slice(half * 512, (half + 1) * 512)
                nc.tensor.matmul(kr0[:, sl], ones[:, 0:P], ktb[0:1, sl],
                                 start=True, stop=True, skip_group_check=True)
            # rows 1-3 via DRAM round-trip broadcasts (zero engine time)
            q01.dma_start(d_kscr[bi][:], ktb)
            krows = [kr0]
            for h in range(1, H):
                kr = persist.tile([P, N], BF, name=f"krow{tag}{h}", tag=f"krow{h}")
                _s = d_kscr[bi][h:h + 1, :]
                sap = bass.AP(tensor=_s.tensor, offset=_s.offset,
                              ap=[[0, P]] + list(_s.ap)[1:])
                (q01 if h < 2 else q23).dma_start(kr, sap)
                krows.append(kr)
            return krows

        def emit_conv(src, cwT, cbt, tag, act_epi=True):
            """oc [cc][128, N] bf16 = relu(grouped 1x1 conv + bias).

            Baseline mechanism: per-group [64, 512] psum, bias+relu fused
            into the DVE epilogue (add, max with per-partition bias)."""
            oc = [persist.tile([P, N], BF, name=f"oc{tag}{cc}", tag=f"oc{cc}",
                               bufs=1) for cc in range(GC)]
            for g in range(4):
                cc, ro = g // 2, (g % 2) * 64
                for half in range(2):
                    sl = slice(half * 512, (half + 1) * 512)
                    pc = pk.tile([64, 512], F32, name="pconv", tag="w")
                    nc.tensor.matmul(pc, cwT[cc][ro:ro + 64, :],
                                     src[cc][ro:ro + 64, sl], start=True, stop=True)
                    nc.vector.tensor_scalar(
                        oc[cc][ro:ro + 64, sl], pc, cbt[ro:ro + 64, cc:cc + 1],
                        0.0, ALU.add, ALU.max)
            return oc

        def emit_cqT(ocs, tag):
            """[jq][128, 256] bf16 = 0.25 * oc[:, jlist]^T (gather+transpose,
            baseline mechanism)."""
            ocq = [persist.tile([P, J], BF, name=f"g{tag}{cc}",
                                tag=f"g{tag}{cc}", bufs=1) for cc in range(GC)]
            for cc in range(GC):
                nc.gpsimd.indirect_copy(ocq[cc], ocs[cc], jidx, True)
            outs = []
            for jq in range(JC):
                pts = pk.tile([P, C], BF, name="ptr", tag="w")
                for cc in range(GC):
                    nc.tensor.transpose(pts[:, cc * P:(cc + 1) * P],
                                        ocq[cc][:, jq * P:(jq + 1) * P], identity)
                t = persist.tile([P, C], BF, name=f"cqT{tag}{jq}",
                                 tag=f"cqT{tag}{jq}", bufs=1)
                nc.vector.tensor_scalar_mul(t, pts, 0.25)
                outs.append(t)
            return outs

        def emit_gather(src, tag):
            """[cc][128, J] bf16 = src[:, jlist] via GpSimd indirect copy."""
            out = [persist.tile([P, J], BF, name=f"g{tag}{cc}", tag=f"g{tag}{cc}",
                                bufs=1) for cc in range(GC)]
            for cc in range(GC):
                nc.gpsimd.indirect_copy(out[cc], src[cc], jidx, True)
            return out

        def emit_message(qts, krows, ocqT, tag, hook=None, warm=False,
                         transposed=False):
            """pm [cc][half][128, 512] f32 psum = sum_j 0.25*oc^T*sig*roi.

            h-major: all jq chunks for head h before moving to h+1, so the
            cc0 psum tiles (heads 0,1) complete ~10us before the cc1 ones,
            and the krow_h broadcast DMAs get a full head-phase of slack.
            hook(h) is called after head h's matmuls are emitted."""
            tags = ["pma0", "pma1", "pmb0", "pmb1"]
            if transposed:
                # 4 tiles, each packing two i-chunks' [128, 256] msg^T
                pms = [[pm.tile([P, 512], F32, name=f"pm{tag}t{t}",
                                tag=tags[t])] for t in range(4)]
            else:
                pms = [[pm.tile([P, 512], F32, name=f"pm{tag}{cc}{hf}",
                                tag=tags[cc * 2 + hf]) for hf in range(2)]
                       for cc in range(GC)]
            for h in range(H):
                for jq in range(JC):
                    s = sp.tile([P, N], BF, name="sig", tag="sig")
                    nc.scalar.activation(s, krows[h], AF.Sigmoid,
                                         bias=qts[jq][:, h:h + 1])
                    a = ap_.tile([P, N], BF, name="attn", tag="attn")
                    eng = nc.vector if h in (0, 3) else nc.gpsimd
                    eng.tensor_mul(a, s, roiTq[jq])
                    if warm and not _CACHE.get("nowarm") and h == H - 1 and jq == JC - 1:
                        sqw = consts.tile([P, 1], F32, name="sqwarm",
                                          tag="sqwarm")
                        nc.scalar.activation(sqw, s[:, 0:1], AF.Sqrt)
                    if transposed:
                        # a as stationary: psum accumulates msg^T [i, ch].
                        # Tiles 0/1 hold the ch 0:128 blocks (heads 0,1) for
                        # ics 0-3 / 4-7; tiles 2/3 the ch 128:256 blocks —
                        # so early reads never touch a still-accumulating
                        # tile (real PSUM forbids read-during-accumulate).
                        for ic in range(NC_):
                            t = (0 if h < 2 else 2) + ic // 4
                            co = (ic % 4) * P + (h % 2) * 64
                            nc.tensor.matmul(
                                pms[t][0][:, co:co + 64],
                                a[:, ic * P:(ic + 1) * P],
                                ocqT[jq][:, h * 64:(h + 1) * 64],
                                start=(jq == 0), stop=(jq == JC - 1),
                                skip_group_check=True)
                    else:
                        for half in range(2):
                            sl = slice(half * 512, (half + 1) * 512)
                            nc.tensor.matmul(
                                pms[h // 2][half][(h % 2) * 64:(h % 2) * 64 + 64, :],
                                ocqT[jq][:, h * 64:(h + 1) * 64], a[:, sl],
                                start=(jq == 0), stop=(jq == JC - 1),
                                skip_group_check=True)
                if hook is not None:
                    hook(h, pms)
            return pms

        # keep the PE busy from t~0 so the HAM clock gate opens before the
        # first real matmuls (cold PE runs at half speed); the pm psum tiles
        # are unused until the message phase and get cleared by start=True.
        pwarm = pm.tile([P, 512], F32, name="pm10", tag="pma0")
        for wi in range(10):
            nc.tensor.matmul(pwarm[:, 0:128], identity, identity,
                             start=True, stop=True, skip_group_check=True)

        # ================= block 1 =================
        krows1 = emit_krow(xT, spatT, w1T, w1sT, 0, "1", nc.gpsimd, dma)
        # stream the remaining roi chunks + grow during phase 1
        for jq in range(2, JC):
            dma.dma_start(roiTq[jq], d_roiTq[jq * P:(jq + 1) * P, :])
        _sm = d_smb[:]
        sm_b = bass.AP(tensor=_sm.tensor, offset=_sm.offset,
                       ap=[[0, P]] + list(_sm.ap))
        growd = consts.tile([P, N], BF, name="growd", tag="growd")
        dma.dma_start(growd, sm_b)
        qts1 = emit_qt(xq, spatq, w1T, w1sT, "1")
        o1c = emit_conv(xT, cw1T, cb1t, "1", act_epi=False)
        o1cqT = emit_cqT(o1c, "1")
        grow = consts.tile([P, N], BF, name="grow", tag="grow")  # 1 + f/4
        nc.vector.tensor_scalar(grow, growd, -0.25, 1.25, ALU.mult, ALU.add)
        fqt = consts.tile([P, NC_], F32, name="fqt", tag="fqt")  # f/4 = .25-.25*sm
        nc.vector.tensor_scalar(fqt, smt, -0.25, 0.25, ALU.mult, ALU.add)
        # t1 = o1c * grow runs early (o1c and grow are ready at ~12us)
        t1 = [persist.tile([P, N], BF, name=f"fin1a{cc}", tag=f"fin1a{cc}", bufs=1)
              for cc in range(GC)]
        nc.vector.tensor_mul(t1[0], o1c[0], grow)
        nc.vector.tensor_mul(t1[1], o1c[1], grow)
        # out1 = t1 + pm1 (pm1 already holds msg/4); the cc0 half and its
        # gather run mid-phase via the h==1 hook (cc0 psums done after h1)
        out1 = [persist.tile([P, N], BF, name=f"out1{cc}", tag=f"out1{cc}")
                for cc in range(GC)]
        out1q = [persist.tile([P, J], BF, name=f"gx2{cc}", tag=f"gx2{cc}",
                              bufs=1) for cc in range(GC)]

        def b1hook(h, pms):
            # cc0 psums are final after h1: assemble out1[0] during the h2
            # phase (DVE); gather it during the h3 phase (Pool is free then,
            # h3 muls run on DVE)
            if h == 1:
                for hf in range(2):
                    sl = slice(hf * 512, (hf + 1) * 512)
                    nc.vector.scalar_tensor_tensor(out1[0][:, sl], pms[0][hf],
                                                   1.0, t1[0][:, sl],
                                                   ALU.mult, ALU.add)
            elif h == 2:
                nc.gpsimd.indirect_copy(out1q[0], out1[0], jidx, True)

        pms1 = emit_message(qts1, krows1, o1cqT, "1", hook=(b1hook if not _CACHE.get("nohook") else None))
        if _CACHE.get("nohook"):
            for hf in range(2):
                sl = slice(hf * 512, (hf + 1) * 512)
                nc.vector.scalar_tensor_tensor(out1[0][:, sl], pms1[0][hf],
                                               1.0, t1[0][:, sl],
                                               ALU.mult, ALU.add)
            nc.gpsimd.indirect_copy(out1q[0], out1[0], jidx, True)
        for hf in range(2):
            sl = slice(hf * 512, (hf + 1) * 512)
            nc.vector.scalar_tensor_tensor(out1[1][:, sl], pms1[1][hf], 1.0,
                                           t1[1][:, sl], ALU.mult, ALU.add)
        nc.gpsimd.indirect_copy(out1q[1], out1[1], jidx, True)

        # ================= block 2 =================
        krows2 = emit_krow(out1, spatT, w2T, w2sT, 1, "2", dma, nc.gpsimd)
        qts2 = emit_qt(out1q, spatq, w2T, w2sT, "2")
        o2c = emit_conv(out1, cw2T, cb2t, "2", act_epi=False)
        o2cqT = emit_cqT(o2c, "2")
        # o2c^T tiles for the final phase, built while messages run
        o2cTs = []
        for ic in range(NC_):
            ptsA = pk.tile([P, C], BF, name="ptA", tag="w")
            for cc in range(GC):
                nc.tensor.transpose(ptsA[:, cc * P:(cc + 1) * P],
                                    o2c[cc][:, ic * P:(ic + 1) * P], identity)
            t = persist.tile([P, C], BF, name=f"o2cTs{ic}", tag=f"o2cTs{ic}")
            nc.vector.tensor_copy(t, ptsA)
            o2cTs.append(t)
        # fold the f-term scale in ahead of time: foc = (f/4) * o2cT, so the
        # per-ic critical path in the finals is a plain bf16 add (2x mode)
        foc = []
        for ic in range(NC_):
            t = persist.tile([P, C], BF, name=f"foc{ic}", tag=f"foc{ic}")
            nc.gpsimd.tensor_scalar(t, o2cTs[ic], fqt[:, ic:ic + 1], 0.0,
                                    ALU.mult, ALU.bypass)
            foc.append(t)
        # split-cc finals: the ch 0:128 (heads 0,1) message psum columns
        # finish a full head-phase before ch 128:256, so half of every LN
        # chain runs during the h2/h3 sigmoid phases. Messages are psum-
        # resident in [i, ch] layout (transposed matmuls) — no copies or
        # transposes needed.
        vt = [persist.tile([P, C], BF, name=f"lnv{ic}", tag=f"lnv{ic}")
              for ic in range(NC_)]
        vstats = [persist.tile([P, 12], F32, name=f"lnst{ic}", tag=f"lnst{ic}")
                  for ic in range(NC_)]

        def final_early(h, pms):
            if h != 1 or _CACHE.get("nohook"):
                return
            for ic in range(NC_):
                co = (ic % 4) * P
                nc.vector.tensor_add(vt[ic][:, 0:P], foc[ic][:, 0:P],
                                     pms[ic // 4][0][:, co:co + P])
                nc.vector.bn_stats(vstats[ic][:, 0:6], vt[ic][:, 0:P])

        pms2 = emit_message(qts2, krows2, o2cqT, "2", hook=final_early,
                            warm=True, transposed=True)

        obuf = [persist.tile([P, 2 * C], BF, name=f"obuf{t}", tag=f"obuf{t}")
                for t in range(NC_ // 2)]
        for ic in range(NC_):
            co = (ic % 4) * P
            if _CACHE.get("nohook"):
                nc.vector.tensor_add(vt[ic][:, 0:P], foc[ic][:, 0:P],
                                     pms2[ic // 4][0][:, co:co + P])
                nc.vector.bn_stats(vstats[ic][:, 0:6], vt[ic][:, 0:P])
            nc.vector.tensor_add(vt[ic][:, P:C], foc[ic][:, P:C],
                                 pms2[2 + ic // 4][0][:, co:co + P])
            nc.vector.bn_stats(vstats[ic][:, 6:12], vt[ic][:, P:C])
            mv = lnp.tile([P, nc.vector.BN_AGGR_DIM], F32, name="lnmv", tag="lnmv")
            nc.vector.bn_aggr(mv, vstats[ic])
            rstd = lnp.tile([P, 1], F32, name="lnrstd", tag="lnrstd")
            nc.scalar.activation(rstd, mv[:, 1:2], AF.Sqrt, bias=epst)
            nc.vector.reciprocal(rstd, rstd)
            w = lnp.tile([P, C], BF, name="lnw", tag="lnw")
            nc.gpsimd.tensor_scalar(w, vt[ic], mv[:, 0:1], rstd,
                                    ALU.subtract, ALU.mult)
            if not ln_trivial:
                nc.gpsimd.tensor_mul(w, w, lngrow)
                nc.gpsimd.tensor_add(w, w, lnbrow)
            o = obuf[ic // 2][:, (ic % 2) * C:(ic % 2) * C + C]
            nc.gpsimd.tensor_add(o, w, o2cTs[ic])
            if ic % 2 == 1:
                _o = d_out[(ic - 1) * P:(ic + 1) * P, :]
                dst = bass.AP(tensor=_o.tensor, offset=_o.offset,
                              ap=[[C, P], [P * C, 2], [1, C]])
                dma.dma_start(dst, obuf[ic // 2])

    nc.finalize()
    return nc


# revision 47
# speedup vs baseline: 1.2097x; 1.0197x over previous
"""Trainium2 Bass kernel for nn_Graph_module_net_0_18631568130110.

GNN message-passing block (two chained masked-sigmoid attention + grouped-conv
layers with a LayerNorm). Shapes: B=8, N=1024, C=MID=OUT=256, h=4, groups=4.

Math simplifications (exact):
  - The reference's relu(cosine)/top-k "present" column mask is the identity
    (unit diagonal of the cosine matrix puts every column in its own row's
    top-k, and the scatter is global), so it is omitted.
  - masks = masks_roi * score_mask[j]: rows j with score_mask[j]==0 contribute
    nothing to the attention message. The kernel compacts the j axis to the
    surviving ~N/2 indices (host-computed index list; selection only, no
    arithmetic on host) and pads to a multiple of 128.

Sharding: data-parallel over batch B across the 8 cores.

Layout strategy per core (J = padded count of surviving j):
  - attention built in [j(part), i(free)] layout; ACT (the only sigmoid
    engine) is the bottleneck at ~1038ns per [128,1024] sigmoid, so ACT does
    almost nothing else: one sigmoid per (j-chunk, head) with per-partition
    bias qt over krow_h = broadcast(kt[:,h]) tiles.
  - krow broadcast tiles are built by a DRAM round-trip (ktb -> internal DRAM
    scratch -> stride-0 partition-broadcast DMAs), zero engine time.
  - conv biases are seeded into PSUM by rank-1 PE matmuls (ones x cb row), so
    conv epilogues are pure relu copies; they run on ACT inside its forced
    idle windows (kernel head / block transition, while krow DMAs fly).
  - the 0.25-scaled transposed conv-output message stationaries (ocqT) are
    computed directly by PE matmuls against the gathered inputs (xq / out1q)
    instead of gather+transpose+copy of the full conv output.
  - all big operands are bf16: PE matmuls 4x faster than fp32, DVE
    elementwise 2x; sigmoid/mask products stay within the 2e-2 tolerance.
  - elementwise mask-mul work is split between DVE and GpSimd.
"""

import numpy as np
from contextlib import ExitStack

import concourse.bass as bass
import concourse.bacc as bacc
import concourse.tile as tile
from concourse import mybir
from concourse.bass_utils import run_bass_kernel_spmd
from concourse.masks import make_identity

F32 = mybir.dt.float32
BF = mybir.dt.bfloat16
U16 = mybir.dt.uint16
AF = mybir.ActivationFunctionType
ALU = mybir.AluOpType
NPBF = mybir.dt.np(BF)

N = 1024
C = 256
H = 4
P = 128
NC_ = N // P          # 8 chunks of 128 nodes
GC = C // P           # 2 partition chunks of channels
EPS_LN = 1e-6

SPK_CB = N + 1024 + 16   # offset of cb rows inside spk (row 0)

_CACHE = {}
TRACE = False


def _build_program(J, ln_trivial):
    JC = J // P
    nc = bacc.Bacc(None, target_bir_lowering=False)
    d_xT = nc.dram_tensor("xT", [C, N], BF, kind="ExternalInput")
    d_xq = nc.dram_tensor("xq", [C, J], BF, kind="ExternalInput")
    d_roiTq = nc.dram_tensor("roiTq", [J, N], BF, kind="ExternalInput")
    d_jidx = nc.dram_tensor("jidx", [P, J // 16], U16, kind="ExternalInput")
    d_smb = nc.dram_tensor("smb", [N], BF, kind="ExternalInput")
    d_bfp = nc.dram_tensor("bfp", [P, 288], BF, kind="ExternalInput")
    d_f32p = nc.dram_tensor("f32p", [P, 20], F32, kind="ExternalInput")
    d_spka = nc.dram_tensor("spka", [2, N + 16 + 512], BF, kind="ExternalInput")
    d_spkq = nc.dram_tensor("spkq", [2, 1024], BF, kind="ExternalInput")
    d_lng = nc.dram_tensor("ln_g", [C], F32, kind="ExternalInput")
    d_lnb = nc.dram_tensor("ln_b", [C], F32, kind="ExternalInput")
    d_kscr = [nc.dram_tensor(f"kscr{b}", [H, N], BF, kind="Internal")
              for b in range(2)]
    d_out = nc.dram_tensor("out", [N, C], BF, kind="ExternalOutput")

    with tile.TileContext(nc) as tc, ExitStack() as ctx:
        consts = ctx.enter_context(tc.tile_pool(name="consts", bufs=1))
        persist = ctx.enter_context(tc.tile_pool(name="persist", bufs=1))
        small = ctx.enter_context(tc.tile_pool(name="small", bufs=1))
        sp = ctx.enter_context(tc.tile_pool(name="sp", bufs=8))
        ap_ = ctx.enter_context(tc.tile_pool(name="ap", bufs=6))
        lnp = ctx.enter_context(tc.tile_pool(name="lnp", bufs=6))
        pm = ctx.enter_context(tc.tile_pool(name="pm", bufs=1, space="PSUM"))
        pk = ctx.enter_context(tc.tile_pool(name="pk", bufs=2, space="PSUM"))
        dma = nc.default_dma_engine   # SP queue

        # ---------------- constants / weights ----------------
        identity = consts.tile([P, P], BF)
        make_identity(nc, identity[:])
        pdum = pk.tile([32, 32], BF, name="pdum", tag="w")
        nc.tensor.transpose(pdum, identity[0:32, 0:32], identity[0:32, 0:32])
        epst = consts.tile([P, 1], F32)
        nc.vector.memset(epst, EPS_LN)

        def load(pool, shape, dt, src, nm, eng=dma):
            t = pool.tile(shape, dt, name=nm, tag=nm)
            eng.dma_start(t, src)
            return t

        # SP queue: xT0 first (gates ktb1), then the small weight packs.
        # Pool queue: xT1 / xq / spkq in parallel.
        xT0 = load(persist, [P, N], BF, d_xT[0:P, :], "xT0")
        spka = load(consts, [2, N + 16 + 512], BF, d_spka[:], "spka",
                    eng=(dma if _CACHE.get("nowarm") else nc.scalar))
        spatT = spka[:, 0:N]
        w1sT = spka[:, N:N + 8]
        w2sT = spka[:, N + 8:N + 16]
        cb1row = spka[0:1, N + 16:N + 272]
        cb2row = spka[0:1, N + 272:N + 528]
        bfp = load(consts, [P, 288], BF, d_bfp[:], "bfp")
        w1T = [bfp[:, 0:8], bfp[:, 8:16]]          # cols 0:4 qT, 4:8 kT
        w2T = [bfp[:, 16:24], bfp[:, 24:32]]
        cw1T = [bfp[:, 32:96], bfp[:, 96:160]]
        cw2T = [bfp[:, 160:224], bfp[:, 224:288]]
        f32p = load(consts, [P, 20], F32, d_f32p[:], "f32p")
        cb1t, cb2t, smt = f32p[:, 0:2], f32p[:, 2:4], f32p[:, 4:12]
        brep = {"1": f32p[:, 12:16], "2": f32p[:, 16:20]}
        jidx = load(consts, [P, J // 16], U16, d_jidx[:], "jidx")
        xT1 = load(persist, [P, N], BF, d_xT[P:C, :], "xT1", eng=nc.gpsimd)
        xT = [xT0, xT1]
        xq = [load(persist, [P, J], BF, d_xq[cc * P:(cc + 1) * P, :], f"xq{cc}",
                   eng=nc.gpsimd) for cc in range(GC)]
        spkq = load(consts, [2, 1024], BF, d_spkq[:], "spkq", eng=nc.gpsimd)
        spatq = spkq[:, 0:J]
        ones = consts.tile([1, 512], BF, name="ones", tag="ones")
        nc.vector.memset(ones, 1.0)
        roiTq = [persist.tile([P, N], BF, name=f"roiTq{jq}", tag=f"roiTq{jq}")
                 for jq in range(JC)]
        for jq in range(2):
            dma.dma_start(roiTq[jq], d_roiTq[jq * P:(jq + 1) * P, :])

        sigwarm = consts.tile([P, 1], F32, name="sigwarm", tag="sigwarm")
        nc.scalar.activation(sigwarm, epst, AF.Sigmoid)

        def bcast_row(dvec, name):
            t = consts.tile([P, C], F32, tag=name)
            _dv = dvec[:]
            ap_b = bass.AP(tensor=_dv.tensor, offset=_dv.offset,
                           ap=[[0, P]] + list(_dv.ap))
            dma.dma_start(t, ap_b)
            return t
        if not ln_trivial:
            lngrow = bcast_row(d_lng, "lngrow")
            lnbrow = bcast_row(d_lnb, "lnbrow")



        # ---------------- per-block emitters ----------------
        def emit_qt(xq_src, spq, wT, wsT, tag):
            """qt[jq] tiles [128, H] f32 (per-partition sigmoid bias)."""
            qts = []
            for jq in range(JC):
                pq = pk.tile([P, H], F32, name="pq", tag="w")
                sl = slice(jq * P, (jq + 1) * P)
                for cc in range(GC):
                    nc.tensor.matmul(pq, xq_src[cc][:, sl], wT[cc][:, 0:H],
                                     start=(cc == 0), stop=False)
                nc.tensor.matmul(pq, spq[:, sl], wsT[:, 0:H], start=False, stop=True)
                qt = small.tile([P, H], F32, name=f"qt{tag}{jq}", tag=f"qt{tag}{jq}")
                nc.vector.tensor_add(qt, pq, brep[tag])
                qts.append(qt)
            return qts

        def emit_krow(x_src, spT, wT, wsT, bi, tag, q01, q23):
            """krow[h] [128, N] bf16 = broadcast(kt[:,h]) via DRAM round-trip.

            q01: queue for the scratch write + h0/h1 broadcasts (on the krow
            critical path); q23: queue for the h2/h3 broadcasts."""
            ktb = small.tile([H, N], BF, name=f"ktb{tag}", tag=f"ktb{tag}")
            for half in range(2):
                pkt = pk.tile([H, 512], F32, name="pkt", tag="w")
                sl = slice(half * 512, (half + 1) * 512)
                for cc in range(GC):
                    nc.tensor.matmul(pkt, wT[cc][:, H:2 * H],
                                     x_src[cc][:, sl], start=(cc == 0), stop=False)
                nc.tensor.matmul(pkt, wsT[:, H:2 * H], spT[:, sl],
                                 start=False, stop=True)
                nc.vector.tensor_copy(ktb[:, sl], pkt)
            # head 0 skips any broadcast: replicate ktb row 0 down the
            # partitions with a rank-1 matmul (ones x row) straight into PSUM
            # (rows 1-3 are not at base partition 0, so only h0 can do this).
            kr0 = pm.tile([P, N], F32, name=f"krp{tag}", tag="krp")
            for half in range(2):
                sl = slice(half * 512, (half + 1) * 512)
                nc.tensor.matmul(kr0[:, sl], ones[:, 0:P], ktb[0:1, sl],
                                 start=True, stop=True, skip_group_check=True)
            # rows 1-3 via DRAM round-trip broadcasts (zero engine time)
            q01.dma_start(d_kscr[bi][:], ktb)
            krows = [kr0]
            for h in range(1, H):
                kr = persist.tile([P, N], BF, name=f"krow{tag}{h}", tag=f"krow{h}")
                _s = d_kscr[bi][h:h + 1, :]
                sap = bass.AP(tensor=_s.tensor, offset=_s.offset,
                              ap=[[0, P]] + list(_s.ap)[1:])
                (q01 if h < 2 else q23).dma_start(kr, sap)
                krows.append(kr)
            return krows

        def emit_conv(src, cwT, cbrow, tag, act_epi=True):
            """oc [cc][128, N] bf16 = relu(grouped 1x1 conv + bias).

            Bias is seeded into PSUM by a rank-1 matmul; the epilogue is a
            single relu copy per paired [128, 512] psum tile."""
            oc = [persist.tile([P, N], BF, name=f"oc{tag}{cc}", tag=f"oc{cc}",
                               bufs=1) for cc in range(GC)]
            for cc in range(GC):
                for half in range(2):
                    sl = slice(half * 512, (half + 1) * 512)
                    pc = pk.tile([P, 512], F32, name="pconv", tag="w")
                    nc.tensor.matmul(pc, cbrow[:, cc * P:(cc + 1) * P], ones,
                                     start=True, stop=False, skip_group_check=True)
                    for ro in (0, 64):
                        nc.tensor.matmul(pc[ro:ro + 64, :], cwT[cc][ro:ro + 64, :],
                                         src[cc][ro:ro + 64, sl],
                                         start=False, stop=True,
                                         skip_group_check=True)
                    nc.vector.tensor_scalar(oc[cc][:, sl], pc, 0.0, 0.0,
                                            ALU.max, ALU.bypass)
            return oc

        def emit_cqT(ocs, tag):
            """[jq][128, 256] bf16 = 0.25 * oc[:, jlist]^T (gather+transpose,
            baseline mechanism)."""
            ocq = [persist.tile([P, J], BF, name=f"g{tag}{cc}",
                                tag=f"g{tag}{cc}", bufs=1) for cc in range(GC)]
            for cc in range(GC):
                nc.gpsimd.indirect_copy(ocq[cc], ocs[cc], jidx, True)
            outs = []
            for jq in range(JC):
                pts = pk.tile([P, C], BF, name="ptr", tag="w")
                for cc in range(GC):
                    nc.tensor.transpose(pts[:, cc * P:(cc + 1) * P],
                                        ocq[cc][:, jq * P:(jq + 1) * P], identity)
                t = persist.tile([P, C], BF, name=f"cqT{tag}{jq}",
                                 tag=f"cqT{tag}{jq}", bufs=1)
                nc.vector.tensor_scalar_mul(t, pts, 0.25)
                outs.append(t)
            return outs

        def emit_gather(src, tag):
            """[cc][128, J] bf16 = src[:, jlist] via GpSimd indirect copy."""
            out = [persist.tile([P, J], BF, name=f"g{tag}{cc}", tag=f"g{tag}{cc}",
                                bufs=1) for cc in range(GC)]
            for cc in range(GC):
                nc.gpsimd.indirect_copy(out[cc], src[cc], jidx, True)
            return out

        def emit_message(qts, krows, ocqT, tag, hook=None, warm=False,
                         transposed=False):
            """pm [cc][half][128, 512] f32 psum = sum_j 0.25*oc^T*sig*roi.

            h-major: all jq chunks for head h before moving to h+1, so the
            cc0 psum tiles (heads 0,1) complete ~10us before the cc1 ones,
            and the krow_h broadcast DMAs get a full head-phase of slack.
            hook(h) is called after head h's matmuls are emitted."""
            tags = ["pma0", "pma1", "pmb0", "pmb1"]
            if transposed:
                # 4 tiles, each packing two i-chunks' [128, 256] msg^T
                pms = [[pm.tile([P, 512], F32, name=f"pm{tag}t{t}",
                                tag=tags[t])] for t in range(4)]
            else:
                pms = [[pm.tile([P, 512], F32, name=f"pm{tag}{cc}{hf}",
                                tag=tags[cc * 2 + hf]) for hf in range(2)]
                       for cc in range(GC)]
            for h in range(H):
                for jq in range(JC):
                    s = sp.tile([P, N], BF, name="sig", tag="sig")
                    nc.scalar.activation(s, krows[h], AF.Sigmoid,
                                         bias=qts[jq][:, h:h + 1])
                    a = ap_.tile([P, N], BF, name="attn", tag="attn")
                    eng = nc.vector if h in (0, 3) else nc.gpsimd
                    eng.tensor_mul(a, s, roiTq[jq])
                    if warm and not _CACHE.get("nowarm") and h == H - 1 and jq == JC - 1:
                        sqw = consts.tile([P, 1], F32, name="sqwarm",
                                          tag="sqwarm")
                        nc.scalar.activation(sqw, s[:, 0:1], AF.Sqrt)
                    if transposed:
                        # a as stationary: psum accumulates msg^T [i, ch].
                        # Tiles 0/1 hold the ch 0:128 blocks (heads 0,1) for
                        # ics 0-3 / 4-7; tiles 2/3 the ch 128:256 blocks —
                        # so early reads never touch a still-accumulating
                        # tile (real PSUM forbids read-during-accumulate).
                        for ic in range(NC_):
                            t = (0 if h < 2 else 2) + ic // 4
                            co = (ic % 4) * P + (h % 2) * 64
                            nc.tensor.matmul(
                                pms[t][0][:, co:co + 64],
                                a[:, ic * P:(ic + 1) * P],
                                ocqT[jq][:, h * 64:(h + 1) * 64],
                                start=(jq == 0), stop=(jq == JC - 1),
                                skip_group_check=True)
                    else:
                        for half in range(2):
                            sl = slice(half * 512, (half + 1) * 512)
                            nc.tensor.matmul(
                                pms[h // 2][half][(h % 2) * 64:(h % 2) * 64 + 64, :],
                                ocqT[jq][:, h * 64:(h + 1) * 64], a[:, sl],
                                start=(jq == 0), stop=(jq == JC - 1),
                                skip_group_check=True)
                if hook is not None:
                    hook(h, pms)
            return pms

        # keep the PE busy from t~0 so the HAM clock gate opens before the
        # first real matmuls (cold PE runs at half speed); the pm psum tiles
        # are unused until the message phase and get cleared by start=True.
        pwarm = pm.tile([P, 512], F32, name="pm10", tag="pma0")
        for wi in range(10):
            nc.tensor.matmul(pwarm[:, 0:128], identity, identity,
                             start=True, stop=True, skip_group_check=True)

        # ================= block 1 =================
        krows1 = emit_krow(xT, spatT, w1T, w1sT, 0, "1", nc.gpsimd, dma)
        # stream the remaining roi chunks + grow during phase 1
        for jq in range(2, JC):
            dma.dma_start(roiTq[jq], d_roiTq[jq * P:(jq + 1) * P, :])
        _sm = d_smb[:]
        sm_b = bass.AP(tensor=_sm.tensor, offset=_sm.offset,
                       ap=[[0, P]] + list(_sm.ap))
        growd = consts.tile([P, N], BF, name="growd", tag="growd")
        dma.dma_start(growd, sm_b)
        qts1 = emit_qt(xq, spatq, w1T, w1sT, "1")
        o1c = emit_conv(xT, cw1T, cb1row, "1", act_epi=False)
        o1cqT = emit_cqT(o1c, "1")
        grow = consts.tile([P, N], BF, name="grow", tag="grow")  # 1 + f/4
        nc.vector.tensor_scalar(grow, growd, -0.25, 1.25, ALU.mult, ALU.add)
        fqt = consts.tile([P, NC_], F32, name="fqt", tag="fqt")  # f/4 = .25-.25*sm
        nc.vector.tensor_scalar(fqt, smt, -0.25, 0.25, ALU.mult, ALU.add)
        # t1 = o1c * grow runs early (o1c and grow are ready at ~12us)
        t1 = [persist.tile([P, N], BF, name=f"fin1a{cc}", tag=f"fin1a{cc}", bufs=1)
              for cc in range(GC)]
        nc.vector.tensor_mul(t1[0], o1c[0], grow)
        nc.vector.tensor_mul(t1[1], o1c[1], grow)
        # out1 = t1 + pm1 (pm1 already holds msg/4); the cc0 half and its
        # gather run mid-phase via the h==1 hook (cc0 psums done after h1)
        out1 = [persist.tile([P, N], BF, name=f"out1{cc}", tag=f"out1{cc}")
                for cc in range(GC)]
        out1q = [persist.tile([P, J], BF, name=f"gx2{cc}", tag=f"gx2{cc}",
                              bufs=1) for cc in range(GC)]

        def b1hook(h, pms):
            # cc0 psums are final after h1: assemble out1[0] during the h2
            # phase (DVE); gather it during the h3 phase (Pool is free then,
            # h3 muls run on DVE)
            if h == 1:
                for hf in range(2):
                    sl = slice(hf * 512, (hf + 1) * 512)
                    nc.vector.scalar_tensor_tensor(out1[0][:, sl], pms[0][hf],
                                                   1.0, t1[0][:, sl],
                                                   ALU.mult, ALU.add)
            elif h == 2:
                nc.gpsimd.indirect_copy(out1q[0], out1[0], jidx, True)

        pms1 = emit_message(qts1, krows1, o1cqT, "1", hook=(b1hook if not _CACHE.get("nohook") else None))
        if _CACHE.get("nohook"):
            for hf in range(2):
                sl = slice(hf * 512, (hf + 1) * 512)
                nc.vector.scalar_tensor_tensor(out1[0][:, sl], pms1[0][hf],
                                               1.0, t1[0][:, sl],
                                               ALU.mult, ALU.add)
            nc.gpsimd.indirect_copy(out1q[0], out1[0], jidx, True)
        for hf in range(2):
            sl = slice(hf * 512, (hf + 1) * 512)
            nc.vector.scalar_tensor_tensor(out1[1][:, sl], pms1[1][hf], 1.0,
                                           t1[1][:, sl], ALU.mult, ALU.add)
        nc.gpsimd.indirect_copy(out1q[1], out1[1], jidx, True)

        # ================= block 2 =================
        krows2 = emit_krow(out1, spatT, w2T, w2sT, 1, "2", dma, nc.gpsimd)
        qts2 = emit_qt(out1q, spatq, w2T, w2sT, "2")
        o2c = emit_conv(out1, cw2T, cb2row, "2", act_epi=False)
        o2cqT = emit_cqT(o2c, "2")
        # o2c^T tiles for the final phase, built while messages run
        o2cTs = []
        for ic in range(NC_):
            ptsA = pk.tile([P, C], BF, name="ptA", tag="w")
            for cc in range(GC):
                nc.tensor.transpose(ptsA[:, cc * P:(cc + 1) * P],
                                    o2c[cc][:, ic * P:(ic + 1) * P], identity)
            t = persist.tile([P, C], BF, name=f"o2cTs{ic}", tag=f"o2cTs{ic}")
            nc.vector.tensor_copy(t, ptsA)
            o2cTs.append(t)
        # fold the f-term scale in ahead of time: foc = (f/4) * o2cT, so the
        # per-ic critical path in the finals is a plain bf16 add (2x mode)
        foc = []
        for ic in range(NC_):
            t = persist.tile([P, C], BF, name=f"foc{ic}", tag=f"foc{ic}")
            nc.gpsimd.tensor_scalar(t, o2cTs[ic], fqt[:, ic:ic + 1], 0.0,
                                    ALU.mult, ALU.bypass)
            foc.append(t)
        # split-cc finals: the ch 0:128 (heads 0,1) message psum columns
        # finish a full head-phase before ch 128:256, so half of every LN
        # chain runs during the h2/h3 sigmoid phases. Messages are psum-
        # resident in [i, ch] layout (transposed matmuls) — no copies or
        # transposes needed.
        vt = [persist.tile([P, C], BF, name=f"lnv{ic}", tag=f"lnv{ic}")
              for ic in range(NC_)]
        vstats = [persist.tile([P, 12], F32, name=f"lnst{ic}", tag=f"lnst{ic}")
                  for ic in range(NC_)]

        def final_early(h, pms):
            if h != 1 or _CACHE.get("nohook"):
                return
            for ic in range(NC_):
                co = (ic % 4) * P
                nc.vector.tensor_add(vt[ic][:, 0:P], foc[ic][:, 0:P],
                                     pms[ic // 4][0][:, co:co + P])
                nc.vector.bn_stats(vstats[ic][:, 0:6], vt[ic][:, 0:P])

        pms2 = emit_message(qts2, krows2, o2cqT, "2", hook=final_early,
                            warm=True, transposed=True)

        obuf = [persist.tile([P, 2 * C], BF, name=f"obuf{t}", tag=f"obuf{t}")
                for t in range(NC_ // 2)]
        for ic in range(NC_):
            co = (ic % 4) * P
            if _CACHE.get("nohook"):
                nc.vector.tensor_add(vt[ic][:, 0:P], foc[ic][:, 0:P],
                                     pms2[ic // 4][0][:, co:co + P])
                nc.vector.bn_stats(vstats[ic][:, 0:6], vt[ic][:, 0:P])
            nc.vector.tensor_add(vt[ic][:, P:C], foc[ic][:, P:C],
                                 pms2[2 + ic // 4][0][:, co:co + P])
            nc.vector.bn_stats(vstats[ic][:, 6:12], vt[ic][:, P:C])
            mv = lnp.tile([P, nc.vector.BN_AGGR_DIM], F32, name="lnmv", tag="lnmv")
            nc.vector.bn_aggr(mv, vstats[ic])
            rstd = lnp.tile([P, 1], F32, name="lnrstd", tag="lnrstd")
            nc.scalar.activation(rstd, mv[:, 1:2], AF.Sqrt, bias=epst)
            nc.vector.reciprocal(rstd, rstd)
            w = lnp.tile([P, C], BF, name="lnw", tag="lnw")
            nc.gpsimd.tensor_scalar(w, vt[ic], mv[:, 0:1], rstd,
                                    ALU.subtract, ALU.mult)
            if not ln_trivial:
                nc.gpsimd.tensor_mul(w, w, lngrow)
                nc.gpsimd.tensor_add(w, w, lnbrow)
            o = obuf[ic // 2][:, (ic % 2) * C:(ic % 2) * C + C]
            nc.gpsimd.tensor_add(o, w, o2cTs[ic])
            if ic % 2 == 1:
                _o = d_out[(ic - 1) * P:(ic + 1) * P, :]
                dst = bass.AP(tensor=_o.tensor, offset=_o.offset,
                              ap=[[C, P], [P * C, 2], [1, C]])
                dma.dma_start(dst, obuf[ic // 2])

    nc.finalize()
    return nc


# revision 49
# speedup vs baseline: 1.2167x; 1.0058x over previous
"""Trainium2 Bass kernel for nn_Graph_module_net_0_18631568130110.

GNN message-passing block (two chained masked-sigmoid attention + grouped-conv
layers with a LayerNorm). Shapes: B=8, N=1024, C=MID=OUT=256, h=4, groups=4.

Math simplifications (exact):
  - The reference's relu(cosine)/top-k "present" column mask is the identity
    (unit diagonal of the cosine matrix puts every column in its own row's
    top-k, and the scatter is global), so it is omitted.
  - masks = masks_roi * score_mask[j]: rows j with score_mask[j]==0 contribute
    nothing to the attention message. The kernel compacts the j axis to the
    surviving ~N/2 indices (host-computed index list; selection only, no
    arithmetic on host) and pads to a multiple of 128.

Sharding: data-parallel over batch B across the 8 cores.

Layout strategy per core (J = padded count of surviving j):
  - attention built in [j(part), i(free)] layout; ACT (the only sigmoid
    engine) is the bottleneck at ~1038ns per [128,1024] sigmoid, so ACT does
    almost nothing else: one sigmoid per (j-chunk, head) with per-partition
    bias qt over krow_h = broadcast(kt[:,h]) tiles.
  - krow broadcast tiles are built by a DRAM round-trip (ktb -> internal DRAM
    scratch -> stride-0 partition-broadcast DMAs), zero engine time.
  - conv biases are seeded into PSUM by rank-1 PE matmuls (ones x cb row), so
    conv epilogues are pure relu copies; they run on ACT inside its forced
    idle windows (kernel head / block transition, while krow DMAs fly).
  - the 0.25-scaled transposed conv-output message stationaries (ocqT) are
    computed directly by PE matmuls against the gathered inputs (xq / out1q)
    instead of gather+transpose+copy of the full conv output.
  - all big operands are bf16: PE matmuls 4x faster than fp32, DVE
    elementwise 2x; sigmoid/mask products stay within the 2e-2 tolerance.
  - elementwise mask-mul work is split between DVE and GpSimd.
"""

import numpy as np
from contextlib import ExitStack

import concourse.bass as bass
import concourse.bacc as bacc
import concourse.tile as tile
from concourse import mybir
from concourse.bass_utils import run_bass_kernel_spmd
from concourse.masks import make_identity

F32 = mybir.dt.float32
BF = mybir.dt.bfloat16
U16 = mybir.dt.uint16
AF = mybir.ActivationFunctionType
ALU = mybir.AluOpType
NPBF = mybir.dt.np(BF)

N = 1024
C = 256
H = 4
P = 128
NC_ = N // P          # 8 chunks of 128 nodes
GC = C // P           # 2 partition chunks of channels
EPS_LN = 1e-6

SPK_CB = N + 1024 + 16   # offset of cb rows inside spk (row 0)

_CACHE = {}
TRACE = False


def _build_program(J, ln_trivial):
    JC = J // P
    nc = bacc.Bacc(None, target_bir_lowering=False)
    d_xT = nc.dram_tensor("xT", [C, N], BF, kind="ExternalInput")
    d_xq = nc.dram_tensor("xq", [C, J], BF, kind="ExternalInput")
    d_roiTq = nc.dram_tensor("roiTq", [J, N], BF, kind="ExternalInput")
    d_jidx = nc.dram_tensor("jidx", [P, J // 16], U16, kind="ExternalInput")
    d_smb = nc.dram_tensor("smb", [N], BF, kind="ExternalInput")
    d_bfp = nc.dram_tensor("bfp", [P, 288], BF, kind="ExternalInput")
    d_f32p = nc.dram_tensor("f32p", [P, 20], F32, kind="ExternalInput")
    d_spka = nc.dram_tensor("spka", [2, N + 16 + 512], BF, kind="ExternalInput")
    d_spkq = nc.dram_tensor("spkq", [2, 1024], BF, kind="ExternalInput")
    d_lng = nc.dram_tensor("ln_g", [C], F32, kind="ExternalInput")
    d_lnb = nc.dram_tensor("ln_b", [C], F32, kind="ExternalInput")
    d_kscr = [nc.dram_tensor(f"kscr{b}", [H, N], BF, kind="Internal")
              for b in range(2)]
    d_out = nc.dram_tensor("out", [N, C], BF, kind="ExternalOutput")

    with tile.TileContext(nc) as tc, ExitStack() as ctx:
        consts = ctx.enter_context(tc.tile_pool(name="consts", bufs=1))
        persist = ctx.enter_context(tc.tile_pool(name="persist", bufs=1))
        small = ctx.enter_context(tc.tile_pool(name="small", bufs=1))
        sp = ctx.enter_context(tc.tile_pool(name="sp", bufs=8))
        ap_ = ctx.enter_context(tc.tile_pool(name="ap", bufs=6))
        lnp = ctx.enter_context(tc.tile_pool(name="lnp", bufs=6))
        pm = ctx.enter_context(tc.tile_pool(name="pm", bufs=1, space="PSUM"))
        pk = ctx.enter_context(tc.tile_pool(name="pk", bufs=2, space="PSUM"))
        dma = nc.default_dma_engine   # SP queue

        # ---------------- constants / weights ----------------
        identity = consts.tile([P, P], BF)
        make_identity(nc, identity[:])
        pdum = pk.tile([32, 32], BF, name="pdum", tag="w")
        nc.tensor.transpose(pdum, identity[0:32, 0:32], identity[0:32, 0:32])
        epst = consts.tile([P, 1], F32)
        nc.vector.memset(epst, EPS_LN)

        def load(pool, shape, dt, src, nm, eng=dma):
            t = pool.tile(shape, dt, name=nm, tag=nm)
            eng.dma_start(t, src)
            return t

        # SP queue: xT0 first (gates ktb1), then the small weight packs.
        # Pool queue: xT1 / xq / spkq in parallel.
        xT0 = load(persist, [P, N], BF, d_xT[0:P, :], "xT0")
        spka = load(consts, [2, N + 16 + 512], BF, d_spka[:], "spka",
                    eng=(dma if _CACHE.get("nowarm") else nc.scalar))
        spatT = spka[:, 0:N]
        w1sT = spka[:, N:N + 8]
        w2sT = spka[:, N + 8:N + 16]
        cb1row = spka[0:1, N + 16:N + 272]
        cb2row = spka[0:1, N + 272:N + 528]
        bfp = load(consts, [P, 288], BF, d_bfp[:], "bfp")
        w1T = [bfp[:, 0:8], bfp[:, 8:16]]          # cols 0:4 qT, 4:8 kT
        w2T = [bfp[:, 16:24], bfp[:, 24:32]]
        cw1T = [bfp[:, 32:96], bfp[:, 96:160]]
        cw2T = [bfp[:, 160:224], bfp[:, 224:288]]
        f32p = load(consts, [P, 20], F32, d_f32p[:], "f32p")
        cb1t, cb2t, smt = f32p[:, 0:2], f32p[:, 2:4], f32p[:, 4:12]
        brep = {"1": f32p[:, 12:16], "2": f32p[:, 16:20]}
        jidx = load(consts, [P, J // 16], U16, d_jidx[:], "jidx")
        xT1 = load(persist, [P, N], BF, d_xT[P:C, :], "xT1", eng=nc.gpsimd)
        xT = [xT0, xT1]
        xq = [load(persist, [P, J], BF, d_xq[cc * P:(cc + 1) * P, :], f"xq{cc}",
                   eng=nc.gpsimd) for cc in range(GC)]
        spkq = load(consts, [2, 1024], BF, d_spkq[:], "spkq", eng=nc.gpsimd)
        spatq = spkq[:, 0:J]
        ones = consts.tile([1, 512], BF, name="ones", tag="ones")
        nc.vector.memset(ones, 1.0)
        roiTq = [persist.tile([P, N], BF, name=f"roiTq{jq}", tag=f"roiTq{jq}")
                 for jq in range(JC)]
        for jq in range(2):
            dma.dma_start(roiTq[jq], d_roiTq[jq * P:(jq + 1) * P, :])

        sigwarm = consts.tile([P, 1], F32, name="sigwarm", tag="sigwarm")
        nc.scalar.activation(sigwarm, epst, AF.Sigmoid)

        def bcast_row(dvec, name):
            t = consts.tile([P, C], F32, tag=name)
            _dv = dvec[:]
            ap_b = bass.AP(tensor=_dv.tensor, offset=_dv.offset,
                           ap=[[0, P]] + list(_dv.ap))
            dma.dma_start(t, ap_b)
            return t
        if not ln_trivial:
            lngrow = bcast_row(d_lng, "lngrow")
            lnbrow = bcast_row(d_lnb, "lnbrow")



        # ---------------- per-block emitters ----------------
        def emit_qt(xq_src, spq, wT, wsT, tag):
            """qt[jq] tiles [128, H] f32 (per-partition sigmoid bias)."""
            qts = []
            for jq in range(JC):
                pq = pk.tile([P, H], F32, name="pq", tag="w")
                sl = slice(jq * P, (jq + 1) * P)
                for cc in range(GC):
                    nc.tensor.matmul(pq, xq_src[cc][:, sl], wT[cc][:, 0:H],
                                     start=(cc == 0), stop=False)
                nc.tensor.matmul(pq, spq[:, sl], wsT[:, 0:H], start=False, stop=True)
                qt = small.tile([P, H], F32, name=f"qt{tag}{jq}", tag=f"qt{tag}{jq}")
                nc.vector.tensor_add(qt, pq, brep[tag])
                qts.append(qt)
            return qts

        def emit_krow(x_src, spT, wT, wsT, bi, tag, q01, q23):
            """krow[h] [128, N] bf16 = broadcast(kt[:,h]) via DRAM round-trip.

            q01: queue for the scratch write + h0/h1 broadcasts (on the krow
            critical path); q23: queue for the h2/h3 broadcasts."""
            ktb = small.tile([H, N], BF, name=f"ktb{tag}", tag=f"ktb{tag}")
            for half in range(2):
                pkt = pk.tile([H, 512], F32, name="pkt", tag="w")
                sl = slice(half * 512, (half + 1) * 512)
                for cc in range(GC):
                    nc.tensor.matmul(pkt, wT[cc][:, H:2 * H],
                                     x_src[cc][:, sl], start=(cc == 0), stop=False)
                nc.tensor.matmul(pkt, wsT[:, H:2 * H], spT[:, sl],
                                 start=False, stop=True)
                nc.vector.tensor_copy(ktb[:, sl], pkt)
            # head 0 skips any broadcast: replicate ktb row 0 down the
            # partitions with a rank-1 matmul (ones x row) straight into PSUM
            # (rows 1-3 are not at base partition 0, so only h0 can do this).
            kr0 = pm.tile([P, N], F32, name=f"krp{tag}", tag="krp")
            for half in range(2):
                sl = slice(half * 512, (half + 1) * 512)
                nc.tensor.matmul(kr0[:, sl], ones[:, 0:P], ktb[0:1, sl],
                                 start=True, stop=True, skip_group_check=True)
            # rows 1-3 via DRAM round-trip broadcasts (zero engine time)
            q01.dma_start(d_kscr[bi][:], ktb)
            krows = [kr0]
            for h in range(1, H):
                kr = persist.tile([P, N], BF, name=f"krow{tag}{h}", tag=f"krow{h}")
                _s = d_kscr[bi][h:h + 1, :]
                sap = bass.AP(tensor=_s.tensor, offset=_s.offset,
                              ap=[[0, P]] + list(_s.ap)[1:])
                (q01 if h < 2 else q23).dma_start(kr, sap)
                krows.append(kr)
            return krows

        def emit_conv(src, cwT, cbrow, tag, act_epi=True):
            """oc [cc][128, N] bf16 = relu(grouped 1x1 conv + bias).

            Bias is seeded into PSUM by a rank-1 matmul; the epilogue is a
            single relu copy per paired [128, 512] psum tile."""
            oc = [persist.tile([P, N], BF, name=f"oc{tag}{cc}", tag=f"oc{cc}",
                               bufs=1) for cc in range(GC)]
            for cc in range(GC):
                for half in range(2):
                    sl = slice(half * 512, (half + 1) * 512)
                    pc = pk.tile([P, 512], F32, name="pconv", tag="w")
                    nc.tensor.matmul(pc, cbrow[:, cc * P:(cc + 1) * P], ones,
                                     start=True, stop=False, skip_group_check=True)
                    for ro in (0, 64):
                        nc.tensor.matmul(pc[ro:ro + 64, :], cwT[cc][ro:ro + 64, :],
                                         src[cc][ro:ro + 64, sl],
                                         start=False, stop=True,
                                         skip_group_check=True)
                    nc.vector.tensor_scalar(oc[cc][:, sl], pc, 0.0, 0.0,
                                            ALU.max, ALU.bypass)
            return oc

        def emit_cqT(ocs, tag):
            """[jq][128, 256] bf16 = 0.25 * oc[:, jlist]^T (gather+transpose,
            baseline mechanism)."""
            ocq = [persist.tile([P, J], BF, name=f"g{tag}{cc}",
                                tag=f"g{tag}{cc}", bufs=1) for cc in range(GC)]
            for cc in range(GC):
                nc.gpsimd.indirect_copy(ocq[cc], ocs[cc], jidx, True)
            outs = []
            for jq in range(JC):
                pts = pk.tile([P, C], BF, name="ptr", tag="w")
                for cc in range(GC):
                    nc.tensor.transpose(pts[:, cc * P:(cc + 1) * P],
                                        ocq[cc][:, jq * P:(jq + 1) * P], identity)
                t = persist.tile([P, C], BF, name=f"cqT{tag}{jq}",
                                 tag=f"cqT{tag}{jq}", bufs=1)
                nc.vector.tensor_scalar_mul(t, pts, 0.25)
                outs.append(t)
            return outs

        def emit_gather(src, tag):
            """[cc][128, J] bf16 = src[:, jlist] via GpSimd indirect copy."""
            out = [persist.tile([P, J], BF, name=f"g{tag}{cc}", tag=f"g{tag}{cc}",
                                bufs=1) for cc in range(GC)]
            for cc in range(GC):
                nc.gpsimd.indirect_copy(out[cc], src[cc], jidx, True)
            return out

        def emit_message(qts, krows, ocqT, tag, hook=None, warm=False,
                         transposed=False):
            """pm [cc][half][128, 512] f32 psum = sum_j 0.25*oc^T*sig*roi.

            h-major: all jq chunks for head h before moving to h+1, so the
            cc0 psum tiles (heads 0,1) complete ~10us before the cc1 ones,
            and the krow_h broadcast DMAs get a full head-phase of slack.
            hook(h) is called after head h's matmuls are emitted."""
            tags = ["pma0", "pma1", "pmb0", "pmb1"]
            if transposed:
                # 4 tiles, each packing two i-chunks' [128, 256] msg^T
                pms = [[pm.tile([P, 512], F32, name=f"pm{tag}t{t}",
                                tag=tags[t])] for t in range(4)]
            else:
                pms = [[pm.tile([P, 512], F32, name=f"pm{tag}{cc}{hf}",
                                tag=tags[cc * 2 + hf]) for hf in range(2)]
                       for cc in range(GC)]
            for h in range(H):
                for jq in range(JC):
                    s = sp.tile([P, N], BF, name="sig", tag="sig")
                    nc.scalar.activation(s, krows[h], AF.Sigmoid,
                                         bias=qts[jq][:, h:h + 1])
                    a = ap_.tile([P, N], BF, name="attn", tag="attn")
                    eng = nc.vector if h in (0, 3) else nc.gpsimd
                    eng.tensor_mul(a, s, roiTq[jq])
                    if warm and not _CACHE.get("nowarm") and h == H - 1 and jq == JC - 1:
                        sqw = consts.tile([P, 1], F32, name="sqwarm",
                                          tag="sqwarm")
                        nc.scalar.activation(sqw, s[:, 0:1], AF.Sqrt)
                    if transposed:
                        # a as stationary: psum accumulates msg^T [i, ch].
                        # Tiles 0/1 hold the ch 0:128 blocks (heads 0,1) for
                        # ics 0-3 / 4-7; tiles 2/3 the ch 128:256 blocks —
                        # so early reads never touch a still-accumulating
                        # tile (real PSUM forbids read-during-accumulate).
                        for ic in range(NC_):
                            t = (0 if h < 2 else 2) + ic // 4
                            co = (ic % 4) * P + (h % 2) * 64
                            nc.tensor.matmul(
                                pms[t][0][:, co:co + 64],
                                a[:, ic * P:(ic + 1) * P],
                                ocqT[jq][:, h * 64:(h + 1) * 64],
                                start=(jq == 0), stop=(jq == JC - 1),
                                skip_group_check=True)
                    else:
                        for half in range(2):
                            sl = slice(half * 512, (half + 1) * 512)
                            nc.tensor.matmul(
                                pms[h // 2][half][(h % 2) * 64:(h % 2) * 64 + 64, :],
                                ocqT[jq][:, h * 64:(h + 1) * 64], a[:, sl],
                                start=(jq == 0), stop=(jq == JC - 1),
                                skip_group_check=True)
                if hook is not None:
                    hook(h, pms)
            return pms

        # keep the PE busy from t~0 so the HAM clock gate opens before the
        # first real matmuls (cold PE runs at half speed); the pm psum tiles
        # are unused until the message phase and get cleared by start=True.
        pwarm = pm.tile([P, 512], F32, name="pm10", tag="pma0")
        for wi in range(10):
            nc.tensor.matmul(pwarm[:, 0:128], identity, identity,
                             start=True, stop=True, skip_group_check=True)

        # ================= block 1 =================
        krows1 = emit_krow(xT, spatT, w1T, w1sT, 0, "1", nc.gpsimd, dma)
        # stream the remaining roi chunks + grow during phase 1
        for jq in range(2, JC):
            dma.dma_start(roiTq[jq], d_roiTq[jq * P:(jq + 1) * P, :])
        _sm = d_smb[:]
        sm_b = bass.AP(tensor=_sm.tensor, offset=_sm.offset,
                       ap=[[0, P]] + list(_sm.ap))
        growd = consts.tile([P, N], BF, name="growd", tag="growd")
        dma.dma_start(growd, sm_b)
        qts1 = emit_qt(xq, spatq, w1T, w1sT, "1")
        o1c = emit_conv(xT, cw1T, cb1row, "1", act_epi=False)
        o1cqT = emit_cqT(o1c, "1")
        grow = consts.tile([P, N], BF, name="grow", tag="grow")  # 1 + f/4
        nc.vector.tensor_scalar(grow, growd, -0.25, 1.25, ALU.mult, ALU.add)
        fqt = consts.tile([P, NC_], F32, name="fqt", tag="fqt")  # f/4 = .25-.25*sm
        nc.vector.tensor_scalar(fqt, smt, -0.25, 0.25, ALU.mult, ALU.add)
        # t1 = o1c * grow runs early (o1c and grow are ready at ~12us)
        t1 = [persist.tile([P, N], BF, name=f"fin1a{cc}", tag=f"fin1a{cc}", bufs=1)
              for cc in range(GC)]
        nc.vector.tensor_mul(t1[0], o1c[0], grow)
        nc.vector.tensor_mul(t1[1], o1c[1], grow)
        # out1 = t1 + pm1 (pm1 already holds msg/4); the cc0 half and its
        # gather run mid-phase via the h==1 hook (cc0 psums done after h1)
        out1 = [persist.tile([P, N], BF, name=f"out1{cc}", tag=f"out1{cc}")
                for cc in range(GC)]
        out1q = [persist.tile([P, J], BF, name=f"gx2{cc}", tag=f"gx2{cc}",
                              bufs=1) for cc in range(GC)]

        def b1hook(h, pms):
            # cc0 psums are final after h1: assemble out1[0] during the h2
            # phase (DVE); gather it during the h3 phase (Pool is free then,
            # h3 muls run on DVE)
            if h == 1:
                for hf in range(2):
                    sl = slice(hf * 512, (hf + 1) * 512)
                    nc.vector.scalar_tensor_tensor(out1[0][:, sl], pms[0][hf],
                                                   1.0, t1[0][:, sl],
                                                   ALU.mult, ALU.add)
            elif h == 2:
                nc.gpsimd.indirect_copy(out1q[0], out1[0], jidx, True)

        pms1 = emit_message(qts1, krows1, o1cqT, "1", hook=(b1hook if not _CACHE.get("nohook") else None))
        if _CACHE.get("nohook"):
            for hf in range(2):
                sl = slice(hf * 512, (hf + 1) * 512)
                nc.vector.scalar_tensor_tensor(out1[0][:, sl], pms1[0][hf],
                                               1.0, t1[0][:, sl],
                                               ALU.mult, ALU.add)
            nc.gpsimd.indirect_copy(out1q[0], out1[0], jidx, True)
        for hf in range(2):
            sl = slice(hf * 512, (hf + 1) * 512)
            nc.vector.scalar_tensor_tensor(out1[1][:, sl], pms1[1][hf], 1.0,
                                           t1[1][:, sl], ALU.mult, ALU.add)
        nc.gpsimd.indirect_copy(out1q[1], out1[1], jidx, True)

        # ================= block 2 =================
        krows2 = emit_krow(out1, spatT, w2T, w2sT, 1, "2", dma, nc.gpsimd)
        qts2 = emit_qt(out1q, spatq, w2T, w2sT, "2")
        o2c = emit_conv(out1, cw2T, cb2row, "2", act_epi=False)
        o2cqT = emit_cqT(o2c, "2")
        # o2c^T tiles for the final phase, built while messages run
        o2cTs = []
        for ic in range(NC_):
            ptsA = pk.tile([P, C], BF, name="ptA", tag="w")
            for cc in range(GC):
                nc.tensor.transpose(ptsA[:, cc * P:(cc + 1) * P],
                                    o2c[cc][:, ic * P:(ic + 1) * P], identity)
            t = persist.tile([P, C], BF, name=f"o2cTs{ic}", tag=f"o2cTs{ic}")
            nc.vector.tensor_copy(t, ptsA)
            o2cTs.append(t)
        # fold the f-term scale in ahead of time: foc = (f/4) * o2cT, so the
        # per-ic critical path in the finals is a plain bf16 add (2x mode)
        foc = []
        for ic in range(NC_):
            t = persist.tile([P, C], BF, name=f"foc{ic}", tag=f"foc{ic}")
            nc.gpsimd.tensor_scalar(t, o2cTs[ic], fqt[:, ic:ic + 1], 0.0,
                                    ALU.mult, ALU.bypass)
            foc.append(t)
        # split-cc finals: the ch 0:128 (heads 0,1) message psum columns
        # finish a full head-phase before ch 128:256, so half of every LN
        # chain runs during the h2/h3 sigmoid phases. Messages are psum-
        # resident in [i, ch] layout (transposed matmuls) — no copies or
        # transposes needed.
        vt = [persist.tile([P, C], BF, name=f"lnv{ic}", tag=f"lnv{ic}")
              for ic in range(NC_)]
        vstats = [persist.tile([P, 12], F32, name=f"lnst{ic}", tag=f"lnst{ic}")
                  for ic in range(NC_)]

        def final_early(h, pms):
            if h != 1 or _CACHE.get("nohook"):
                return
            for ic in range(NC_):
                co = (ic % 4) * P
                nc.vector.tensor_add(vt[ic][:, 0:P], foc[ic][:, 0:P],
                                     pms[ic // 4][0][:, co:co + P])
                nc.vector.bn_stats(vstats[ic][:, 0:6], vt[ic][:, 0:P])

        pms2 = emit_message(qts2, krows2, o2cqT, "2", hook=final_early,
                            warm=True, transposed=True)

        obuf = [persist.tile([P, 2 * C], BF, name=f"obuf{t}", tag=f"obuf{t}")
                for t in range(NC_ // 2)]
        for ic in range(NC_):
            co = (ic % 4) * P
            if _CACHE.get("nohook"):
                nc.vector.tensor_add(vt[ic][:, 0:P], foc[ic][:, 0:P],
                                     pms2[ic // 4][0][:, co:co + P])
                nc.vector.bn_stats(vstats[ic][:, 0:6], vt[ic][:, 0:P])
            nc.vector.tensor_add(vt[ic][:, P:C], foc[ic][:, P:C],
                                 pms2[2 + ic // 4][0][:, co:co + P])
            nc.vector.bn_stats(vstats[ic][:, 6:12], vt[ic][:, P:C])
            mv = lnp.tile([P, nc.vector.BN_AGGR_DIM], F32, name="lnmv", tag="lnmv")
            nc.vector.bn_aggr(mv, vstats[ic])
            rstd = lnp.tile([P, 1], F32, name="lnrstd", tag="lnrstd")
            nc.scalar.activation(rstd, mv[:, 1:2], AF.Sqrt, bias=epst)
            nc.vector.reciprocal(rstd, rstd)
            w = lnp.tile([P, C], BF, name="lnw", tag="lnw")
            weng = nc.gpsimd if ic < 6 else nc.vector
            weng.tensor_scalar(w, vt[ic], mv[:, 0:1], rstd,
                               ALU.subtract, ALU.mult)
            if not ln_trivial:
                weng.tensor_mul(w, w, lngrow)
                weng.tensor_add(w, w, lnbrow)
            o = obuf[ic // 2][:, (ic % 2) * C:(ic % 2) * C + C]
            weng.tensor_add(o, w, o2cTs[ic])
            if ic % 2 == 1:
                _o = d_out[(ic - 1) * P:(ic + 1) * P, :]
                dst = bass.AP(tensor=_o.tensor, offset=_o.offset,
                              ap=[[C, P], [P * C, 2], [1, C]])
                dma.dma_start(dst, obuf[ic // 2])

    nc.finalize()
    return nc
